# revision 25
# baseline (speedup 1.0000x reference)
"""FAVOR+ (Performer) linear attention on 8 Trainium2 NeuronCores.

Math (per batch b, head h, with m = hd = 64, scale = hd**-0.25):
  qkv = x @ W_qkv.T ; q,k,v : [N, H, hd]
  phi(z) = exp(scale*z @ rfs[h] - 0.5*|scale*z|^2)          (z = q or k)
  causal scan:  S_t = S_{t-1} + phi_k[t] (x) v[t] ; z_t = z_{t-1} + phi_k[t]
                out[t] = (phi_q[t] @ S_t) / (phi_q[t] . z_t + 1e-16)

Sharding: data-parallel over batch B=8, one batch per core.

Key tricks vs the naive formulation:
  * rfs is orthogonal*sqrt(hd), so |z|^2 = |z @ rfs|^2 / hd.  This lets the
    host fold scale*rfs into W_q/W_k (P = Wphi @ x^T directly gives the
    random-feature projection) and the kernel recovers the -0.5|z|^2 term
    from P itself: exponent = P - |P|^2/(2*hd), via a block-diagonal
    (-1/128) matmul on P^2.
  * All matmul operands are bf16 (host-rounded; PSUM accumulation stays
    fp32), running the PE at full rate — fp32 operands cost 4 cycles/row.
  * v is stored interleaved with a ones column per head ([V|1], stride 65)
    so the intra-chunk numerator+denominator and the state update are one
    matmul each per head.
  * The scan state [S|z] accumulates in a single PSUM bank across all 32
    chunks (exact fp32); a bf16 SBUF shadow feeds the inter-chunk matmul.

Per-core chunked formulation (chunk L=128 tokens):
  AT   = phi_k_chunk @ phi_q_chunk^T          [j, i]  (PE, feature-major)
  ATm  = AT * triu_mask (keep j <= i)                 (DVE multiply, 4 heads/op)
  num' = phi_q @ [S | z]  +  ATm^T @ [V | 1]  [i, 65] (PE, PSUM-accumulated)
  S   += phi_k^T @ [V | 1]                            (PE, PSUM-resident)
  out  = num'[:, :64] / (num'[:, 64] + 1e-16)         (DVE recip + bcast mul)
"""

import os
import numpy as np

# sim-safe variant closes every PSUM group before reads (CoreSim rejects
# mid-accumulation-group PSUM reads; hardware does not care).  Slightly more
# DVE work.  Used to validate logic in CoreSim.
SIM_SAFE = bool(int(os.environ.get("KERNEL_SIM_SAFE", "0")))

# debug bisect stage: proj | vproj | tmk | scan_nostate | scan_state | full
STAGE = os.environ.get("KERNEL_STAGE", "full")

B, N, C, H = 8, 4096, 768, 12
HD = 64
G = H // 2            # head pairs stacked on 128 partitions
NCH = 512             # tokens per outer chunk
NSUB = NCH // 128     # 128-token scan chunks per outer chunk
NBIG = N // NCH
NCHUNK = N // 128     # 32 scan chunks
SCALE = HD ** -0.25

_CACHE = {}


def _build_bass():
    import concourse.bass as bass
    import concourse.mybir as mybir
    import concourse.tile as tile
    from concourse import bacc
    from contextlib import ExitStack

    f32 = mybir.dt.float32
    bf16 = mybir.dt.bfloat16
    AF = mybir.ActivationFunctionType

    nc = bacc.Bacc("TRN2", target_bir_lowering=False)
    xT = nc.declare_dram_parameter("xT", [C, N], bf16, isOutput=False)
    WT = nc.declare_dram_parameter("WT", [C, 3 * C], bf16, isOutput=False)
    nhalf = nc.declare_dram_parameter("nhalf", [128, 128], bf16, isOutput=False)
    maskT = nc.declare_dram_parameter("maskT", [128, 128], bf16, isOutput=False)
    ident = nc.declare_dram_parameter("ident", [128, 128], bf16, isOutput=False)
    outd = nc.declare_dram_parameter("out", [N, C], f32, isOutput=True)

    with tile.TileContext(nc) as tc, ExitStack() as ctx:
        consts = ctx.enter_context(tc.tile_pool(name="consts", bufs=1))
        xt_p = ctx.enter_context(tc.tile_pool(name="xt", bufs=2))
        sq_p = ctx.enter_context(tc.tile_pool(name="sq", bufs=2))
        phi_p = ctx.enter_context(tc.tile_pool(name="phi", bufs=2))
        tm_p = ctx.enter_context(tc.tile_pool(name="tm", bufs=2))
        v_p = ctx.enter_context(tc.tile_pool(name="v", bufs=2))
        atm_p = ctx.enter_context(tc.tile_pool(name="atm", bufs=4))
        stb_p = ctx.enter_context(tc.tile_pool(name="stb", bufs=1))
        stf_p = ctx.enter_context(tc.tile_pool(name="stf", bufs=2))
        den_p = ctx.enter_context(tc.tile_pool(name="den", bufs=4))
        out_p = ctx.enter_context(tc.tile_pool(name="outp", bufs=2))

        pp_ps = ctx.enter_context(tc.tile_pool(name="pp", bufs=2, space="PSUM"))
        num_ps = ctx.enter_context(tc.tile_pool(name="nm", bufs=2, space="PSUM"))
        pa_ps = ctx.enter_context(tc.tile_pool(name="pa", bufs=2, space="PSUM"))
        st_ps = ctx.enter_context(tc.tile_pool(name="stp", bufs=1, space="PSUM"))

        # ---- constants ----
        wt = []
        for ct in range(6):
            t = consts.tile([128, 3 * C], bf16, tag=f"wt{ct}")
            nc.sync.dma_start(out=t[:], in_=WT[ct * 128:(ct + 1) * 128, :])
            wt.append(t)
        nh_sb = consts.tile([128, 128], bf16, tag="nh")
        nc.sync.dma_start(out=nh_sb[:], in_=nhalf[:])
        mk_sb = consts.tile([128, 128], bf16, tag="mk")
        nc.sync.dma_start(out=mk_sb[:], in_=maskT[:])
        idb_sb = consts.tile([128, 128], bf16, tag="idb")
        nc.sync.dma_start(out=idb_sb[:], in_=ident[:])
        zb = consts.tile([128, 1], f32, tag="zb")
        nc.vector.memset(zb[:], 0.0)

        # state [S|z] per head: pair g -> cols g*65..g*65+65, head parity e ->
        # partitions e*64..e*64+64.  One PSUM bank, accumulated over all 32
        # chunks; stbf is the bf16 SBUF shadow used by the inter-chunk matmul.
        st = None
        if not SIM_SAFE:
            st = []
            for b in range(2):
                stb_t = st_ps.tile([128, 512], f32, tag=f"st{b}")
                st.append(stb_t)
        stf = [None, None]  # fp32 SBUF state chain (sim-safe variant only)
        # state shadow, K=128 zero-padded: pair g cols g*130..g*130+130, head
        # parity e owns cols +e*65..+e*65+65 on partitions e*64..e*64+64, the
        # other parity's partitions stay zero so inter-chunk matmuls can run
        # with K=128 at tile_position (0,0) (uniform with the intra matmuls;
        # mixing row tile_positions inside one PSUM group faults the device).
        stbf_tiles = []
        for i in range(2):
            t = stb_p.tile([128, G * 130], bf16, tag=f"stb{i}")
            nc.vector.memset(t[:], 0.0)
            stbf_tiles.append(t)
        stbf = None

        # xt prefetch for chunk 0
        xt_tiles = []
        for ct in range(6):
            t = xt_p.tile([128, NCH], bf16, tag=f"xt{ct}")
            nc.gpsimd.dma_start(out=t[:], in_=xT[ct * 128:(ct + 1) * 128, 0:NCH])
            xt_tiles.append(t)

        for cb in range(NBIG):
            n0 = cb * NCH
            xt = xt_tiles

            # ---- P projection (feature-major random features for q,k) ----
            # P^T[f, n] accumulates in pf; then pf += nhalf @ (P*P) adds the
            # -|P|^2/128 exponent term; exp() gives phi^T in bf16.
            # Software-pipelined so the nhalf matmul (which waits on the Act
            # square) sits behind the next tile's projection matmuls.
            phi = [None] * 12
            pend = None  # (pf, sq, ft) awaiting nhalf+exp
            for ft in range(13):
                if ft < 12:
                    pf = pp_ps.tile([128, NCH], f32, tag="pp")
                    for ct in range(6):
                        nc.tensor.matmul(
                            pf[:], wt[ct][:, ft * 128:(ft + 1) * 128],
                            xt[ct][:],
                            start=(ct == 0), stop=(SIM_SAFE and ct == 5))
                    sq = sq_p.tile([128, NCH], bf16, tag="sqr")
                    nc.scalar.square(sq[:], pf[:])
                else:
                    pf = sq = None
                if pend is not None:
                    ppf, psq, pft = pend
                    if SIM_SAFE:
                        pn2 = pa_ps.tile([128, NCH], f32, tag="pa")
                        nc.tensor.matmul(pn2[:], nh_sb[:], psq[:],
                                         start=True, stop=True)
                        esum = stf_p.tile([128, NCH], f32, tag="esum")
                        nc.scalar.copy(esum[:], ppf[:])
                        nc.vector.tensor_add(esum[:], esum[:], pn2[:])
                        esrc = esum
                    else:
                        nc.tensor.matmul(ppf[:], nh_sb[:], psq[:],
                                         start=False, stop=True)
                        esrc = ppf
                    t = phi_p.tile([128, NCH], bf16, tag=f"ph{pft}")
                    nc.scalar.activation(t[:], esrc[:], AF.Exp, bias=zb[:])
                    phi[pft] = t
                pend = (pf, sq, ft) if ft < 12 else None
            phiq, phik = phi[:6], phi[6:]

            if STAGE == "proj":
                ot = out_p.tile([128, C], f32, tag="out")
                nc.vector.tensor_copy(ot[:, :512], phi[0][:])
                nc.vector.tensor_copy(ot[:, :512], phi[6][:])
                nc.sync.dma_start(out=outd[n0:n0 + 128, :], in_=ot[:])
                continue

            # ---- v projection (token-major), interleaved [V|1] stride 65 ----
            vsb = []
            for nt in range(NSUB):
                t = v_p.tile([128, H * 65], bf16, tag=f"v{nt}")
                ones_v = t[:].rearrange("p (h d) -> p h d", d=65)[:, :, 64:65]
                nc.vector.memset(ones_v, 1.0)
                for half in range(2):
                    pv = pp_ps.tile([128, NCH], f32, tag="pp")
                    fsl = slice(2 * C + half * 384, 2 * C + (half + 1) * 384)
                    for ct in range(6):
                        nc.tensor.matmul(
                            pv[:, :384], xt[ct][:, nt * 128:(nt + 1) * 128],
                            wt[ct][:, fsl], start=(ct == 0), stop=(ct == 5))
                    dst = t[:, half * 390:(half + 1) * 390].rearrange(
                        "p (h d) -> p h d", d=65)[:, :, 0:64]
                    src = pv[:, :384].rearrange("p (h d) -> p h d", d=64)
                    nc.scalar.copy(dst, src)
                vsb.append(t)

            if STAGE == "vproj":
                ot = out_p.tile([128, C], f32, tag="out")
                nc.vector.tensor_copy(ot[:], vsb[0][:, :C])
                nc.sync.dma_start(out=outd[n0:n0 + 128, :], in_=ot[:])
                continue

            # ---- phi_k token-major: transpose via regular matmul against the
            # identity (bf16 PE-transpose into PSUM is broken on HW; a plain
            # matmul phi_k^T @ I costs the same and lands in fp32 PSUM).
            # 4 sub-chunk transposes share one bank, one Act copy per pair.
            tmk = []
            for g in range(G):
                ptr = pp_ps.tile([128, 512], f32, tag="pp")
                for sub in range(NSUB):
                    nc.tensor.matmul(
                        ptr[:, sub * 128:(sub + 1) * 128],
                        phik[g][:, sub * 128:(sub + 1) * 128], idb_sb[:],
                        start=(sub == 0), stop=(sub == NSUB - 1))
                t = tm_p.tile([128, 512], bf16, tag=f"tm{g}")
                nc.scalar.copy(t[:], ptr[:])
                tmk.append(t)

            if STAGE == "tmk":
                ot = out_p.tile([128, C], f32, tag="out")
                nc.vector.tensor_copy(ot[:, :512], tmk[0][:])
                nc.sync.dma_start(out=outd[n0:n0 + 128, :], in_=ot[:])
                continue

            # ---- prefetch next chunk's x^T while the scan runs ----
            if cb + 1 < NBIG:
                xt_tiles = []
                for ct in range(6):
                    t = xt_p.tile([128, NCH], bf16, tag=f"xt{ct}")
                    nc.gpsimd.dma_start(
                        out=t[:],
                        in_=xT[ct * 128:(ct + 1) * 128,
                               (cb + 1) * NCH:(cb + 2) * NCH])
                    xt_tiles.append(t)

            # ---- causal scan over 128-token chunks ----
            for sub in range(NSUB):
                ci = cb * NSUB + sub
                ssl = slice(sub * 128, (sub + 1) * 128)
                first = (ci == 0)
                last = (ci == NCHUNK - 1)
                nums = []
                for _grp in range(2):
                    pn = num_ps.tile([128, 512], f32, tag="num")
                    nums.append(pn)

                inter_on = STAGE == "full"
                state_on = STAGE in ("full", "scan_state")
                # inter-chunk: phi_q @ [S | z], one matmul per head pair
                # (zero-padded shadow keeps heads separate at K=128, and both
                # heads' 65-col blocks are adjacent in the num bank)
                if not first and inter_on:
                    for g in range(G):
                        nc.tensor.matmul(
                            nums[g // 3][:, (2 * g % 6) * 65:
                                         (2 * g % 6) * 65 + 130],
                            phiq[g][:, ssl],
                            stbf[:, g * 130:g * 130 + 130],
                            start=(g % 3 == 0), stop=False)

                # AT = phi_k @ phi_q^T.  Heads grouped by partition parity
                # so each PSUM bank group has ONE tile_position; one masked
                # DVE multiply per bank.
                atm_slot = {}
                for heads, e in (((0, 2, 4, 6), 0), ((8, 10), 0),
                                 ((1, 3, 5, 7), 1), ((9, 11), 1)):
                    nh_ = len(heads)
                    esl = slice(e * 64, (e + 1) * 64)
                    pa = pa_ps.tile([128, 512], f32, tag="pa")
                    for si, h in enumerate(heads):
                        g = h // 2
                        nc.tensor.matmul(
                            pa[:, si * 128:(si + 1) * 128],
                            phik[g][esl, ssl], phiq[g][esl, ssl],
                            start=(si == 0), stop=(si == nh_ - 1),
                            tile_position=(e * 64, 0))
                    atm = atm_p.tile([128, 512], bf16, tag="atm")
                    nc.vector.tensor_mul(
                        atm[:, :nh_ * 128].rearrange(
                            "p (h m) -> p h m", m=128),
                        pa[:, :nh_ * 128].rearrange("p (h m) -> p h m", m=128),
                        mk_sb[:].rearrange("p (o m) -> p o m", o=1)
                        .broadcast_to((128, nh_, 128)))
                    for si, h in enumerate(heads):
                        atm_slot[h] = (atm, si)

                if STAGE == "scan_at":
                    ot = out_p.tile([128, C], f32, tag="out")
                    nc.vector.tensor_copy(ot[:, :512], atm_slot[0][0][:])
                    nc.sync.dma_start(
                        out=outd[n0 + sub * 128:n0 + (sub + 1) * 128, :],
                        in_=ot[:])
                    continue

                # state update: S += phi_k^T @ [V | 1] (PSUM-resident group
                # spanning all 32 chunks; per-chunk groups + SBUF fp32 chain
                # in the sim-safe variant)
                stc = st
                if SIM_SAFE:
                    stc = []
                    for b in range(2):
                        stc_t = st_ps.tile([128, 512], f32, tag=f"st{b}")
                        stc.append(stc_t)
                for g in range(G if state_on else 0):
                    bk, gg = g // 3, g % 3
                    nc.tensor.matmul(
                        stc[bk][:, gg * 130:gg * 130 + 130],
                        tmk[g][:, sub * 128:(sub + 1) * 128],
                        vsb[sub][:, 2 * g * 65:2 * g * 65 + 130],
                        start=(gg == 0 if SIM_SAFE else (first and gg == 0)),
                        stop=(gg == 2 if SIM_SAFE else (last and gg == 2)),
                        skip_group_check=True)
                if not last and state_on:
                    stbf_new = stbf_tiles[ci % 2]
                    if SIM_SAFE:
                        ssrc = []
                        for b in range(2):
                            stf_new = stf_p.tile([128, 390], f32,
                                                 tag=f"stf{b}")
                            if first:
                                nc.vector.tensor_copy(stf_new[:],
                                                      stc[b][:, :390])
                            else:
                                nc.vector.tensor_add(stf_new[:],
                                                     stc[b][:, :390],
                                                     stf[b][:])
                            stf[b] = stf_new
                            ssrc.append(stf_new)
                    else:
                        ssrc = st
                    for b in range(2):
                        for e in range(2):
                            esl = slice(e * 64, (e + 1) * 64)
                            csl = slice(e * 65, e * 65 + 65)
                            dst = stbf_new[esl, b * 390:(b + 1) * 390]                                .rearrange("p (g c) -> p g c", c=130)[:, :, csl]
                            nc.scalar.copy(
                                dst, ssrc[b][esl, :390].rearrange(
                                    "p (g c) -> p g c", c=130)[:, :, csl])
                    stbf = stbf_new

                # intra-chunk: ATm^T @ [V | 1]  (closes each num bank)
                for h in range(H):
                    hh = h % 6
                    atm_t, si = atm_slot[h]
                    nc.tensor.matmul(
                        nums[h // 6][:, hh * 65:hh * 65 + 65],
                        atm_t[:, si * 128:(si + 1) * 128],
                        vsb[sub][:, h * 65:h * 65 + 65],
                        start=((first or not inter_on) and hh == 0),
                        stop=(hh == 5))

                if STAGE == "scan_intra":
                    ot = out_p.tile([128, C], f32, tag="out")
                    nc.scalar.copy(ot[:, :384], nums[0][:, :384])
                    nc.scalar.copy(ot[:, 384:768], nums[1][:, :384])
                    nc.sync.dma_start(
                        out=outd[n0 + sub * 128:n0 + (sub + 1) * 128, :],
                        in_=ot[:])
                    continue

                # ---- normalize and store ----
                den = den_p.tile([128, H], f32, tag="den")
                for grp in range(2):
                    src = nums[grp][:, :390].rearrange(
                        "p (h d) -> p h d", d=65)[:, :, 64:65]
                    dst = den[:, grp * 6:(grp + 1) * 6].rearrange(
                        "p (h o) -> p h o", o=1)
                    nc.scalar.activation(dst, src, AF.Copy, bias=1e-16)
                nc.vector.reciprocal(den[:], den[:])
                ot = out_p.tile([128, C], f32, tag="out")
                for grp in range(2):
                    src = nums[grp][:, :390].rearrange(
                        "p (h d) -> p h d", d=65)[:, :, 0:64]
                    rec = den[:, grp * 6:(grp + 1) * 6].rearrange(
                        "p (h o) -> p h o", o=1).broadcast_to((128, 6, 64))
                    dst = ot[:, grp * 384:(grp + 1) * 384].rearrange(
                        "p (h d) -> p h d", d=64)
                    nc.vector.tensor_mul(dst, src, rec)
                nc.sync.dma_start(
                    out=outd[n0 + sub * 128:n0 + (sub + 1) * 128, :],
                    in_=ot[:])

    if not nc.is_finalized():
        nc.finalize()
    return nc


def _host_inputs(x, W_qkv, rfs):
    import ml_dtypes

    bf16 = ml_dtypes.bfloat16
    x = np.asarray(x, dtype=np.float32)
    W = np.asarray(W_qkv, dtype=np.float64)
    rfs = np.asarray(rfs, dtype=np.float64)

    # Fold scale*rfs into the q/k projection weights: P = Wphi @ x^T is then
    # the random-feature projection directly (rfs orthogonality gives
    # |z|^2 = |P|^2/64, recovered on-chip).
    Wq = W[:C].reshape(H, HD, C)
    Wk = W[C:2 * C].reshape(H, HD, C)
    Wphi_q = np.einsum('hlm,hlc->hmc', rfs * SCALE, Wq).reshape(C, C)
    Wphi_k = np.einsum('hlm,hlc->hmc', rfs * SCALE, Wk).reshape(C, C)
    Wall = np.concatenate([Wphi_q, Wphi_k, W[2 * C:]], axis=0)  # [3C, C]
    WT = np.ascontiguousarray(Wall.T).astype(bf16)              # [C, 3C]

    nhalf = np.zeros((128, 128), np.float32)
    nhalf[:64, :64] = -1.0 / (2 * HD)
    nhalf[64:, 64:] = -1.0 / (2 * HD)
    maskT = np.triu(np.ones((128, 128), np.float32))    # keep j <= i
    ident = np.eye(128, dtype=np.float32)

    shared = {"WT": WT, "nhalf": nhalf.astype(bf16),
              "maskT": maskT.astype(bf16), "ident": ident.astype(bf16)}
    in_maps = []
    for b in range(B):
        m = {"xT": np.ascontiguousarray(x[b].T).astype(bf16)}
        m.update(shared)
        in_maps.append(m)
    return in_maps


def kernel(x, W_qkv, rfs):
    from concourse.bass_utils import run_bass_kernel_spmd

    if "nc" not in _CACHE:
        _CACHE["nc"] = _build_bass()
    nc = _CACHE["nc"]
    in_maps = _host_inputs(x, W_qkv, rfs)
    res = run_bass_kernel_spmd(nc, in_maps, list(range(B)))
    return np.stack([res.results[b]["out"] for b in range(B)], axis=0)


# revision 27
# speedup vs baseline: 1.0138x; 1.0138x over previous
"""FAVOR+ (Performer) linear attention on 8 Trainium2 NeuronCores.

Math (per batch b, head h, with m = hd = 64, scale = hd**-0.25):
  qkv = x @ W_qkv.T ; q,k,v : [N, H, hd]
  phi(z) = exp(scale*z @ rfs[h] - 0.5*|scale*z|^2)          (z = q or k)
  causal scan:  S_t = S_{t-1} + phi_k[t] (x) v[t] ; z_t = z_{t-1} + phi_k[t]
                out[t] = (phi_q[t] @ S_t) / (phi_q[t] . z_t + 1e-16)

Sharding: data-parallel over batch B=8, one batch per core.

Key tricks vs the naive formulation:
  * rfs is orthogonal*sqrt(hd), so |z|^2 = |z @ rfs|^2 / hd.  This lets the
    host fold scale*rfs into W_q/W_k (P = Wphi @ x^T directly gives the
    random-feature projection) and the kernel recovers the -0.5|z|^2 term
    from P itself: exponent = P - |P|^2/(2*hd), via a block-diagonal
    (-1/128) matmul on P^2.
  * All matmul operands are bf16 (host-rounded; PSUM accumulation stays
    fp32), running the PE at full rate — fp32 operands cost 4 cycles/row.
  * v is stored interleaved with a ones column per head ([V|1], stride 65)
    so the intra-chunk numerator+denominator and the state update are one
    matmul each per head.
  * The scan state [S|z] accumulates in a single PSUM bank across all 32
    chunks (exact fp32); a bf16 SBUF shadow feeds the inter-chunk matmul.

Per-core chunked formulation (chunk L=128 tokens):
  AT   = phi_k_chunk @ phi_q_chunk^T          [j, i]  (PE, feature-major)
  ATm  = AT * triu_mask (keep j <= i)                 (DVE multiply, 4 heads/op)
  num' = phi_q @ [S | z]  +  ATm^T @ [V | 1]  [i, 65] (PE, PSUM-accumulated)
  S   += phi_k^T @ [V | 1]                            (PE, PSUM-resident)
  out  = num'[:, :64] / (num'[:, 64] + 1e-16)         (DVE recip + bcast mul)
"""

import os
import numpy as np

# sim-safe variant closes every PSUM group before reads (CoreSim rejects
# mid-accumulation-group PSUM reads; hardware does not care).  Slightly more
# DVE work.  Used to validate logic in CoreSim.
SIM_SAFE = bool(int(os.environ.get("KERNEL_SIM_SAFE", "0")))

# debug bisect stage: proj | vproj | tmk | scan_nostate | scan_state | full
STAGE = os.environ.get("KERNEL_STAGE", "full")

B, N, C, H = 8, 4096, 768, 12
HD = 64
G = H // 2            # head pairs stacked on 128 partitions
NCH = 512             # tokens per outer chunk
NSUB = NCH // 128     # 128-token scan chunks per outer chunk
NBIG = N // NCH
NCHUNK = N // 128     # 32 scan chunks
SCALE = HD ** -0.25

_CACHE = {}


def _build_bass():
    import concourse.bass as bass
    import concourse.mybir as mybir
    import concourse.tile as tile
    from concourse import bacc
    from contextlib import ExitStack

    f32 = mybir.dt.float32
    bf16 = mybir.dt.bfloat16
    AF = mybir.ActivationFunctionType

    nc = bacc.Bacc("TRN2", target_bir_lowering=False)
    xT = nc.declare_dram_parameter("xT", [C, N], bf16, isOutput=False)
    WT = nc.declare_dram_parameter("WT", [C, 3 * C], bf16, isOutput=False)
    nhalf = nc.declare_dram_parameter("nhalf", [128, 128], bf16, isOutput=False)
    maskT = nc.declare_dram_parameter("maskT", [128, 128], bf16, isOutput=False)
    ident = nc.declare_dram_parameter("ident", [128, 128], bf16, isOutput=False)
    outd = nc.declare_dram_parameter("out", [N, C], f32, isOutput=True)

    with tile.TileContext(nc) as tc, ExitStack() as ctx:
        consts = ctx.enter_context(tc.tile_pool(name="consts", bufs=1))
        xt_p = ctx.enter_context(tc.tile_pool(name="xt", bufs=2))
        sq_p = ctx.enter_context(tc.tile_pool(name="sq", bufs=2))
        phi_p = ctx.enter_context(tc.tile_pool(name="phi", bufs=2))
        tm_p = ctx.enter_context(tc.tile_pool(name="tm", bufs=2))
        v_p = ctx.enter_context(tc.tile_pool(name="v", bufs=2))
        atm_p = ctx.enter_context(tc.tile_pool(name="atm", bufs=8))
        stb_p = ctx.enter_context(tc.tile_pool(name="stb", bufs=1))
        stf_p = ctx.enter_context(tc.tile_pool(name="stf", bufs=2))
        den_p = ctx.enter_context(tc.tile_pool(name="den", bufs=4))
        out_p = ctx.enter_context(tc.tile_pool(name="outp", bufs=2))

        pp_ps = ctx.enter_context(tc.tile_pool(name="pp", bufs=2, space="PSUM"))
        num_ps = ctx.enter_context(tc.tile_pool(name="nm", bufs=2, space="PSUM"))
        pa_ps = ctx.enter_context(tc.tile_pool(name="pa", bufs=2, space="PSUM"))
        st_ps = ctx.enter_context(tc.tile_pool(name="stp", bufs=1, space="PSUM"))

        # ---- constants ----
        wt = []
        for ct in range(6):
            t = consts.tile([128, 3 * C], bf16, tag=f"wt{ct}")
            nc.sync.dma_start(out=t[:], in_=WT[ct * 128:(ct + 1) * 128, :])
            wt.append(t)
        nh_sb = consts.tile([128, 128], bf16, tag="nh")
        nc.sync.dma_start(out=nh_sb[:], in_=nhalf[:])
        mk_sb = consts.tile([128, 128], bf16, tag="mk")
        nc.sync.dma_start(out=mk_sb[:], in_=maskT[:])
        idb_sb = consts.tile([128, 128], bf16, tag="idb")
        nc.sync.dma_start(out=idb_sb[:], in_=ident[:])
        zb = consts.tile([128, 1], f32, tag="zb")
        nc.vector.memset(zb[:], 0.0)

        # state [S|z] per head: pair g -> cols g*65..g*65+65, head parity e ->
        # partitions e*64..e*64+64.  One PSUM bank, accumulated over all 32
        # chunks; stbf is the bf16 SBUF shadow used by the inter-chunk matmul.
        st = None
        if not SIM_SAFE:
            st = []
            for b in range(2):
                stb_t = st_ps.tile([128, 512], f32, tag=f"st{b}")
                st.append(stb_t)
        stf = [None, None]  # fp32 SBUF state chain (sim-safe variant only)
        # state shadow, K=128 zero-padded: pair g cols g*130..g*130+130, head
        # parity e owns cols +e*65..+e*65+65 on partitions e*64..e*64+64, the
        # other parity's partitions stay zero so inter-chunk matmuls can run
        # with K=128 at tile_position (0,0) (uniform with the intra matmuls;
        # mixing row tile_positions inside one PSUM group faults the device).
        stbf_tiles = []
        for i in range(2):
            t = stb_p.tile([128, G * 130], bf16, tag=f"stb{i}")
            nc.vector.memset(t[:], 0.0)
            stbf_tiles.append(t)
        stbf = None

        # xt prefetch for chunk 0
        xt_tiles = []
        for ct in range(6):
            t = xt_p.tile([128, NCH], bf16, tag=f"xt{ct}")
            nc.gpsimd.dma_start(out=t[:], in_=xT[ct * 128:(ct + 1) * 128, 0:NCH])
            xt_tiles.append(t)

        for cb in range(NBIG):
            n0 = cb * NCH
            xt = xt_tiles

            # ---- P projection (feature-major random features for q,k) ----
            # P^T[f, n] accumulates in pf; then pf += nhalf @ (P*P) adds the
            # -|P|^2/128 exponent term; exp() gives phi^T in bf16.
            # Software-pipelined so the nhalf matmul (which waits on the Act
            # square) sits behind the next tile's projection matmuls.
            phi = [None] * 12
            pend = None  # (pf, sq, ft) awaiting nhalf+exp
            for ft in range(13):
                if ft < 12:
                    pf = pp_ps.tile([128, NCH], f32, tag="pp")
                    for ct in range(6):
                        nc.tensor.matmul(
                            pf[:], wt[ct][:, ft * 128:(ft + 1) * 128],
                            xt[ct][:],
                            start=(ct == 0), stop=(SIM_SAFE and ct == 5))
                    sq = sq_p.tile([128, NCH], bf16, tag="sqr")
                    nc.scalar.square(sq[:], pf[:])
                else:
                    pf = sq = None
                if pend is not None:
                    ppf, psq, pft = pend
                    if SIM_SAFE:
                        pn2 = pa_ps.tile([128, NCH], f32, tag="pa")
                        nc.tensor.matmul(pn2[:], nh_sb[:], psq[:],
                                         start=True, stop=True)
                        esum = stf_p.tile([128, NCH], f32, tag="esum")
                        nc.scalar.copy(esum[:], ppf[:])
                        nc.vector.tensor_add(esum[:], esum[:], pn2[:])
                        esrc = esum
                    else:
                        nc.tensor.matmul(ppf[:], nh_sb[:], psq[:],
                                         start=False, stop=True)
                        esrc = ppf
                    t = phi_p.tile([128, NCH], bf16, tag=f"ph{pft}")
                    nc.scalar.activation(t[:], esrc[:], AF.Exp, bias=zb[:])
                    phi[pft] = t
                pend = (pf, sq, ft) if ft < 12 else None
            phiq, phik = phi[:6], phi[6:]

            if STAGE == "proj":
                ot = out_p.tile([128, C], f32, tag="out")
                nc.vector.tensor_copy(ot[:, :512], phi[0][:])
                nc.vector.tensor_copy(ot[:, :512], phi[6][:])
                nc.sync.dma_start(out=outd[n0:n0 + 128, :], in_=ot[:])
                continue

            # ---- v projection (token-major), interleaved [V|1] stride 65 ----
            vsb = []
            for nt in range(NSUB):
                t = v_p.tile([128, H * 65], bf16, tag=f"v{nt}")
                ones_v = t[:].rearrange("p (h d) -> p h d", d=65)[:, :, 64:65]
                nc.vector.memset(ones_v, 1.0)
                for half in range(2):
                    pv = pp_ps.tile([128, NCH], f32, tag="pp")
                    fsl = slice(2 * C + half * 384, 2 * C + (half + 1) * 384)
                    for ct in range(6):
                        nc.tensor.matmul(
                            pv[:, :384], xt[ct][:, nt * 128:(nt + 1) * 128],
                            wt[ct][:, fsl], start=(ct == 0), stop=(ct == 5))
                    dst = t[:, half * 390:(half + 1) * 390].rearrange(
                        "p (h d) -> p h d", d=65)[:, :, 0:64]
                    src = pv[:, :384].rearrange("p (h d) -> p h d", d=64)
                    nc.scalar.copy(dst, src)
                vsb.append(t)

            if STAGE == "vproj":
                ot = out_p.tile([128, C], f32, tag="out")
                nc.vector.tensor_copy(ot[:], vsb[0][:, :C])
                nc.sync.dma_start(out=outd[n0:n0 + 128, :], in_=ot[:])
                continue

            # ---- phi_k token-major: transpose via regular matmul against the
            # identity (bf16 PE-transpose into PSUM is broken on HW; a plain
            # matmul phi_k^T @ I costs the same and lands in fp32 PSUM).
            # 4 sub-chunk transposes share one bank, one Act copy per pair.
            tmk = []
            for g in range(G):
                ptr = pp_ps.tile([128, 512], f32, tag="pp")
                for sub in range(NSUB):
                    nc.tensor.matmul(
                        ptr[:, sub * 128:(sub + 1) * 128],
                        phik[g][:, sub * 128:(sub + 1) * 128], idb_sb[:],
                        start=(sub == 0), stop=(sub == NSUB - 1))
                t = tm_p.tile([128, 512], bf16, tag=f"tm{g}")
                nc.scalar.copy(t[:], ptr[:])
                tmk.append(t)

            if STAGE == "tmk":
                ot = out_p.tile([128, C], f32, tag="out")
                nc.vector.tensor_copy(ot[:, :512], tmk[0][:])
                nc.sync.dma_start(out=outd[n0:n0 + 128, :], in_=ot[:])
                continue

            # ---- prefetch next chunk's x^T while the scan runs ----
            if cb + 1 < NBIG:
                xt_tiles = []
                for ct in range(6):
                    t = xt_p.tile([128, NCH], bf16, tag=f"xt{ct}")
                    nc.gpsimd.dma_start(
                        out=t[:],
                        in_=xT[ct * 128:(ct + 1) * 128,
                               (cb + 1) * NCH:(cb + 2) * NCH])
                    xt_tiles.append(t)

            # ---- causal scan over 128-token chunks ----
            # AT = phi_k @ phi_q^T.  Heads grouped by partition parity so
            # each PSUM bank group has ONE tile_position; one masked DVE
            # multiply per bank.  Emitted one sub-chunk AHEAD so the intra
            # matmuls never wait on the DVE multiply.
            def at_banks(s):
                sl = slice(s * 128, (s + 1) * 128)
                slot = {}
                for heads, e in (((0, 2, 4, 6), 0), ((8, 10), 0),
                                 ((1, 3, 5, 7), 1), ((9, 11), 1)):
                    nh_ = len(heads)
                    esl = slice(e * 64, (e + 1) * 64)
                    pa = pa_ps.tile([128, 512], f32, tag="pa")
                    for si, h in enumerate(heads):
                        g = h // 2
                        nc.tensor.matmul(
                            pa[:, si * 128:(si + 1) * 128],
                            phik[g][esl, sl], phiq[g][esl, sl],
                            start=(si == 0), stop=(si == nh_ - 1),
                            tile_position=(e * 64, 0))
                    atm = atm_p.tile([128, 512], bf16, tag="atm")
                    nc.vector.tensor_mul(
                        atm[:, :nh_ * 128].rearrange(
                            "p (h m) -> p h m", m=128),
                        pa[:, :nh_ * 128].rearrange("p (h m) -> p h m", m=128),
                        mk_sb[:].rearrange("p (o m) -> p o m", o=1)
                        .broadcast_to((128, nh_, 128)))
                    for si, h in enumerate(heads):
                        slot[h] = (atm, si)
                return slot

            atm_cur = at_banks(0)
            for sub in range(NSUB):
                ci = cb * NSUB + sub
                ssl = slice(sub * 128, (sub + 1) * 128)
                first = (ci == 0)
                last = (ci == NCHUNK - 1)
                nums = []
                for _grp in range(2):
                    pn = num_ps.tile([128, 512], f32, tag="num")
                    nums.append(pn)

                inter_on = STAGE == "full"
                state_on = STAGE in ("full", "scan_state")
                # inter-chunk: phi_q @ [S | z], one matmul per head pair
                # (zero-padded shadow keeps heads separate at K=128, and both
                # heads' 65-col blocks are adjacent in the num bank)
                if not first and inter_on:
                    for g in range(G):
                        nc.tensor.matmul(
                            nums[g // 3][:, (2 * g % 6) * 65:
                                         (2 * g % 6) * 65 + 130],
                            phiq[g][:, ssl],
                            stbf[:, g * 130:g * 130 + 130],
                            start=(g % 3 == 0), stop=False)

                if STAGE == "scan_at":
                    ot = out_p.tile([128, C], f32, tag="out")
                    nc.vector.tensor_copy(ot[:, :512], atm_cur[0][0][:])
                    nc.sync.dma_start(
                        out=outd[n0 + sub * 128:n0 + (sub + 1) * 128, :],
                        in_=ot[:])
                    continue

                # state update: S += phi_k^T @ [V | 1] (PSUM-resident group
                # spanning all 32 chunks; per-chunk groups + SBUF fp32 chain
                # in the sim-safe variant)
                stc = st
                if SIM_SAFE:
                    stc = []
                    for b in range(2):
                        stc_t = st_ps.tile([128, 512], f32, tag=f"st{b}")
                        stc.append(stc_t)
                for g in range(G if state_on else 0):
                    bk, gg = g // 3, g % 3
                    nc.tensor.matmul(
                        stc[bk][:, gg * 130:gg * 130 + 130],
                        tmk[g][:, sub * 128:(sub + 1) * 128],
                        vsb[sub][:, 2 * g * 65:2 * g * 65 + 130],
                        start=(gg == 0 if SIM_SAFE else (first and gg == 0)),
                        stop=(gg == 2 if SIM_SAFE else (last and gg == 2)),
                        skip_group_check=True)
                if not last and state_on:
                    stbf_new = stbf_tiles[ci % 2]
                    if SIM_SAFE:
                        ssrc = []
                        for b in range(2):
                            stf_new = stf_p.tile([128, 390], f32,
                                                 tag=f"stf{b}")
                            if first:
                                nc.vector.tensor_copy(stf_new[:],
                                                      stc[b][:, :390])
                            else:
                                nc.vector.tensor_add(stf_new[:],
                                                     stc[b][:, :390],
                                                     stf[b][:])
                            stf[b] = stf_new
                            ssrc.append(stf_new)
                    else:
                        ssrc = st
                    for b in range(2):
                        for e in range(2):
                            esl = slice(e * 64, (e + 1) * 64)
                            csl = slice(e * 65, e * 65 + 65)
                            dst = stbf_new[esl, b * 390:(b + 1) * 390]                                .rearrange("p (g c) -> p g c", c=130)[:, :, csl]
                            nc.vector.tensor_copy(
                                dst, ssrc[b][esl, :390].rearrange(
                                    "p (g c) -> p g c", c=130)[:, :, csl])
                    stbf = stbf_new

                atm_slot = atm_cur
                if sub + 1 < NSUB:
                    atm_cur = at_banks(sub + 1)

                # intra-chunk: ATm^T @ [V | 1]  (closes each num bank)
                for h in range(H):
                    hh = h % 6
                    atm_t, si = atm_slot[h]
                    nc.tensor.matmul(
                        nums[h // 6][:, hh * 65:hh * 65 + 65],
                        atm_t[:, si * 128:(si + 1) * 128],
                        vsb[sub][:, h * 65:h * 65 + 65],
                        start=((first or not inter_on) and hh == 0),
                        stop=(hh == 5))

                if STAGE == "scan_intra":
                    ot = out_p.tile([128, C], f32, tag="out")
                    nc.scalar.copy(ot[:, :384], nums[0][:, :384])
                    nc.scalar.copy(ot[:, 384:768], nums[1][:, :384])
                    nc.sync.dma_start(
                        out=outd[n0 + sub * 128:n0 + (sub + 1) * 128, :],
                        in_=ot[:])
                    continue

                # ---- normalize and store ----
                den = den_p.tile([128, H], f32, tag="den")
                for grp in range(2):
                    src = nums[grp][:, :390].rearrange(
                        "p (h d) -> p h d", d=65)[:, :, 64:65]
                    dst = den[:, grp * 6:(grp + 1) * 6].rearrange(
                        "p (h o) -> p h o", o=1)
                    nc.scalar.activation(dst, src, AF.Copy, bias=1e-16)
                nc.vector.reciprocal(den[:], den[:])
                ot = out_p.tile([128, C], f32, tag="out")
                for grp in range(2):
                    src = nums[grp][:, :390].rearrange(
                        "p (h d) -> p h d", d=65)[:, :, 0:64]
                    rec = den[:, grp * 6:(grp + 1) * 6].rearrange(
                        "p (h o) -> p h o", o=1).broadcast_to((128, 6, 64))
                    dst = ot[:, grp * 384:(grp + 1) * 384].rearrange(
                        "p (h d) -> p h d", d=64)
                    nc.vector.tensor_mul(dst, src, rec)
                nc.sync.dma_start(
                    out=outd[n0 + sub * 128:n0 + (sub + 1) * 128, :],
                    in_=ot[:])

    if not nc.is_finalized():
        nc.finalize()
    return nc


def _host_inputs(x, W_qkv, rfs):
    import ml_dtypes

    bf16 = ml_dtypes.bfloat16
    x = np.asarray(x, dtype=np.float32)
    W = np.asarray(W_qkv, dtype=np.float64)
    rfs = np.asarray(rfs, dtype=np.float64)

    # Fold scale*rfs into the q/k projection weights: P = Wphi @ x^T is then
    # the random-feature projection directly (rfs orthogonality gives
    # |z|^2 = |P|^2/64, recovered on-chip).
    Wq = W[:C].reshape(H, HD, C)
    Wk = W[C:2 * C].reshape(H, HD, C)
    Wphi_q = np.einsum('hlm,hlc->hmc', rfs * SCALE, Wq).reshape(C, C)
    Wphi_k = np.einsum('hlm,hlc->hmc', rfs * SCALE, Wk).reshape(C, C)
    Wall = np.concatenate([Wphi_q, Wphi_k, W[2 * C:]], axis=0)  # [3C, C]
    WT = np.ascontiguousarray(Wall.T).astype(bf16)              # [C, 3C]

    nhalf = np.zeros((128, 128), np.float32)
    nhalf[:64, :64] = -1.0 / (2 * HD)
    nhalf[64:, 64:] = -1.0 / (2 * HD)
    maskT = np.triu(np.ones((128, 128), np.float32))    # keep j <= i
    ident = np.eye(128, dtype=np.float32)

    shared = {"WT": WT, "nhalf": nhalf.astype(bf16),
              "maskT": maskT.astype(bf16), "ident": ident.astype(bf16)}
    in_maps = []
    for b in range(B):
        m = {"xT": np.ascontiguousarray(x[b].T).astype(bf16)}
        m.update(shared)
        in_maps.append(m)
    return in_maps


def kernel(x, W_qkv, rfs):
    from concourse.bass_utils import run_bass_kernel_spmd

    if "nc" not in _CACHE:
        _CACHE["nc"] = _build_bass()
    nc = _CACHE["nc"]
    in_maps = _host_inputs(x, W_qkv, rfs)
    res = run_bass_kernel_spmd(nc, in_maps, list(range(B)))
    return np.stack([res.results[b]["out"] for b in range(B)], axis=0)


# revision 28
# speedup vs baseline: 1.0338x; 1.0197x over previous
"""FAVOR+ (Performer) linear attention on 8 Trainium2 NeuronCores.

Math (per batch b, head h, with m = hd = 64, scale = hd**-0.25):
  qkv = x @ W_qkv.T ; q,k,v : [N, H, hd]
  phi(z) = exp(scale*z @ rfs[h] - 0.5*|scale*z|^2)          (z = q or k)
  causal scan:  S_t = S_{t-1} + phi_k[t] (x) v[t] ; z_t = z_{t-1} + phi_k[t]
                out[t] = (phi_q[t] @ S_t) / (phi_q[t] . z_t + 1e-16)

Sharding: data-parallel over batch B=8, one batch per core.

Key tricks vs the naive formulation:
  * rfs is orthogonal*sqrt(hd), so |z|^2 = |z @ rfs|^2 / hd.  This lets the
    host fold scale*rfs into W_q/W_k (P = Wphi @ x^T directly gives the
    random-feature projection) and the kernel recovers the -0.5|z|^2 term
    from P itself: exponent = P - |P|^2/(2*hd), via a block-diagonal
    (-1/128) matmul on P^2.
  * All matmul operands are bf16 (host-rounded; PSUM accumulation stays
    fp32), running the PE at full rate — fp32 operands cost 4 cycles/row.
  * v is stored interleaved with a ones column per head ([V|1], stride 65)
    so the intra-chunk numerator+denominator and the state update are one
    matmul each per head.
  * The scan state [S|z] accumulates in a single PSUM bank across all 32
    chunks (exact fp32); a bf16 SBUF shadow feeds the inter-chunk matmul.

Per-core chunked formulation (chunk L=128 tokens):
  AT   = phi_k_chunk @ phi_q_chunk^T          [j, i]  (PE, feature-major)
  ATm  = AT * triu_mask (keep j <= i)                 (DVE multiply, 4 heads/op)
  num' = phi_q @ [S | z]  +  ATm^T @ [V | 1]  [i, 65] (PE, PSUM-accumulated)
  S   += phi_k^T @ [V | 1]                            (PE, PSUM-resident)
  out  = num'[:, :64] / (num'[:, 64] + 1e-16)         (DVE recip + bcast mul)
"""

import os
import numpy as np

# sim-safe variant closes every PSUM group before reads (CoreSim rejects
# mid-accumulation-group PSUM reads; hardware does not care).  Slightly more
# DVE work.  Used to validate logic in CoreSim.
SIM_SAFE = bool(int(os.environ.get("KERNEL_SIM_SAFE", "0")))

# debug bisect stage: proj | vproj | tmk | scan_nostate | scan_state | full
STAGE = os.environ.get("KERNEL_STAGE", "full")

B, N, C, H = 8, 4096, 768, 12
HD = 64
G = H // 2            # head pairs stacked on 128 partitions
NCH = 512             # tokens per outer chunk
NSUB = NCH // 128     # 128-token scan chunks per outer chunk
NBIG = N // NCH
NCHUNK = N // 128     # 32 scan chunks
SCALE = HD ** -0.25

_CACHE = {}


def _build_bass():
    import concourse.bass as bass
    import concourse.mybir as mybir
    import concourse.tile as tile
    from concourse import bacc
    from contextlib import ExitStack

    f32 = mybir.dt.float32
    bf16 = mybir.dt.bfloat16
    AF = mybir.ActivationFunctionType

    nc = bacc.Bacc("TRN2", target_bir_lowering=False)
    xT = nc.declare_dram_parameter("xT", [C, N], bf16, isOutput=False)
    WT = nc.declare_dram_parameter("WT", [C, 3 * C], bf16, isOutput=False)
    nhalf = nc.declare_dram_parameter("nhalf", [128, 128], bf16, isOutput=False)
    maskT = nc.declare_dram_parameter("maskT", [128, 128], bf16, isOutput=False)
    ident = nc.declare_dram_parameter("ident", [128, 128], bf16, isOutput=False)
    outd = nc.declare_dram_parameter("out", [N, C], f32, isOutput=True)

    with tile.TileContext(nc) as tc, ExitStack() as ctx:
        consts = ctx.enter_context(tc.tile_pool(name="consts", bufs=1))
        xt_p = ctx.enter_context(tc.tile_pool(name="xt", bufs=2))
        sq_p = ctx.enter_context(tc.tile_pool(name="sq", bufs=2))
        phi_p = ctx.enter_context(tc.tile_pool(name="phi", bufs=2))
        tm_p = ctx.enter_context(tc.tile_pool(name="tm", bufs=2))
        v_p = ctx.enter_context(tc.tile_pool(name="v", bufs=2))
        atm_p = ctx.enter_context(tc.tile_pool(name="atm", bufs=8))
        stb_p = ctx.enter_context(tc.tile_pool(name="stb", bufs=1))
        stf_p = ctx.enter_context(tc.tile_pool(name="stf", bufs=2))
        den_p = ctx.enter_context(tc.tile_pool(name="den", bufs=4))
        out_p = ctx.enter_context(tc.tile_pool(name="outp", bufs=2))

        pp_ps = ctx.enter_context(tc.tile_pool(name="pp", bufs=2, space="PSUM"))
        num_ps = ctx.enter_context(tc.tile_pool(name="nm", bufs=2, space="PSUM"))
        pa_ps = ctx.enter_context(tc.tile_pool(name="pa", bufs=2, space="PSUM"))
        st_ps = ctx.enter_context(tc.tile_pool(name="stp", bufs=1, space="PSUM"))

        # ---- constants ----
        wt = []
        for ct in range(6):
            t = consts.tile([128, 3 * C], bf16, tag=f"wt{ct}")
            nc.sync.dma_start(out=t[:], in_=WT[ct * 128:(ct + 1) * 128, :])
            wt.append(t)
        nh_sb = consts.tile([128, 128], bf16, tag="nh")
        nc.sync.dma_start(out=nh_sb[:], in_=nhalf[:])
        mk_sb = consts.tile([128, 128], bf16, tag="mk")
        nc.sync.dma_start(out=mk_sb[:], in_=maskT[:])
        idb_sb = consts.tile([128, 128], bf16, tag="idb")
        nc.sync.dma_start(out=idb_sb[:], in_=ident[:])
        zb = consts.tile([128, 1], f32, tag="zb")
        nc.vector.memset(zb[:], 0.0)

        # state [S|z] per head: pair g -> cols g*65..g*65+65, head parity e ->
        # partitions e*64..e*64+64.  One PSUM bank, accumulated over all 32
        # chunks; stbf is the bf16 SBUF shadow used by the inter-chunk matmul.
        st = None
        if not SIM_SAFE:
            st = []
            for b in range(2):
                stb_t = st_ps.tile([128, 512], f32, tag=f"st{b}")
                st.append(stb_t)
        stf = [None, None]  # fp32 SBUF state chain (sim-safe variant only)
        # state shadow, K=128 zero-padded: pair g cols g*130..g*130+130, head
        # parity e owns cols +e*65..+e*65+65 on partitions e*64..e*64+64, the
        # other parity's partitions stay zero so inter-chunk matmuls can run
        # with K=128 at tile_position (0,0) (uniform with the intra matmuls;
        # mixing row tile_positions inside one PSUM group faults the device).
        stbf_tiles = []
        for i in range(2):
            t = stb_p.tile([128, G * 130], bf16, tag=f"stb{i}")
            nc.vector.memset(t[:], 0.0)
            stbf_tiles.append(t)
        stbf = None

        # xt prefetch for chunk 0
        xt_tiles = []
        for ct in range(6):
            t = xt_p.tile([128, NCH], bf16, tag=f"xt{ct}")
            nc.gpsimd.dma_start(out=t[:], in_=xT[ct * 128:(ct + 1) * 128, 0:NCH])
            xt_tiles.append(t)

        for cb in range(NBIG):
            n0 = cb * NCH
            xt = xt_tiles

            # ---- P projection (feature-major random features for q,k) ----
            # P^T[f, n] accumulates in pf; then pf += nhalf @ (P*P) adds the
            # -|P|^2/128 exponent term; exp() gives phi^T in bf16.
            # Software-pipelined so the nhalf matmul (which waits on the Act
            # square) sits behind the next tile's projection matmuls.
            phi = [None] * 12
            pend = None  # (pf, sq, ft) awaiting nhalf+exp
            for ft in range(13):
                if ft < 12:
                    pf = pp_ps.tile([128, NCH], f32, tag="pp")
                    for ct in range(6):
                        nc.tensor.matmul(
                            pf[:], wt[ct][:, ft * 128:(ft + 1) * 128],
                            xt[ct][:],
                            start=(ct == 0), stop=(SIM_SAFE and ct == 5))
                    sq = sq_p.tile([128, NCH], bf16, tag="sqr")
                    nc.scalar.square(sq[:], pf[:])
                else:
                    pf = sq = None
                if pend is not None:
                    ppf, psq, pft = pend
                    if SIM_SAFE:
                        pn2 = pa_ps.tile([128, NCH], f32, tag="pa")
                        nc.tensor.matmul(pn2[:], nh_sb[:], psq[:],
                                         start=True, stop=True)
                        esum = stf_p.tile([128, NCH], f32, tag="esum")
                        nc.scalar.copy(esum[:], ppf[:])
                        nc.vector.tensor_add(esum[:], esum[:], pn2[:])
                        esrc = esum
                    else:
                        nc.tensor.matmul(ppf[:], nh_sb[:], psq[:],
                                         start=False, stop=True)
                        esrc = ppf
                    t = phi_p.tile([128, NCH], bf16, tag=f"ph{pft}")
                    nc.scalar.activation(t[:], esrc[:], AF.Exp, bias=zb[:])
                    phi[pft] = t
                pend = (pf, sq, ft) if ft < 12 else None
            phiq, phik = phi[:6], phi[6:]

            if STAGE == "proj":
                ot = out_p.tile([128, C], f32, tag="out")
                nc.vector.tensor_copy(ot[:, :512], phi[0][:])
                nc.vector.tensor_copy(ot[:, :512], phi[6][:])
                nc.sync.dma_start(out=outd[n0:n0 + 128, :], in_=ot[:])
                continue

            # ---- v projection (token-major), interleaved [V|1] stride 65 ----
            vsb = []
            for nt in range(NSUB):
                t = v_p.tile([128, H * 65], bf16, tag=f"v{nt}")
                ones_v = t[:].rearrange("p (h d) -> p h d", d=65)[:, :, 64:65]
                nc.vector.memset(ones_v, 1.0)
                for half in range(2):
                    pv = pp_ps.tile([128, NCH], f32, tag="pp")
                    fsl = slice(2 * C + half * 384, 2 * C + (half + 1) * 384)
                    for ct in range(6):
                        nc.tensor.matmul(
                            pv[:, :384], xt[ct][:, nt * 128:(nt + 1) * 128],
                            wt[ct][:, fsl], start=(ct == 0), stop=(ct == 5))
                    dst = t[:, half * 390:(half + 1) * 390].rearrange(
                        "p (h d) -> p h d", d=65)[:, :, 0:64]
                    src = pv[:, :384].rearrange("p (h d) -> p h d", d=64)
                    nc.scalar.copy(dst, src)
                vsb.append(t)

            if STAGE == "vproj":
                ot = out_p.tile([128, C], f32, tag="out")
                nc.vector.tensor_copy(ot[:], vsb[0][:, :C])
                nc.sync.dma_start(out=outd[n0:n0 + 128, :], in_=ot[:])
                continue

            # ---- phi_k token-major: transpose via regular matmul against the
            # identity (bf16 PE-transpose into PSUM is broken on HW; a plain
            # matmul phi_k^T @ I costs the same and lands in fp32 PSUM).
            # 4 sub-chunk transposes share one bank, one Act copy per pair.
            tmk = []
            for g in range(G):
                ptr = pp_ps.tile([128, 512], f32, tag="pp")
                for sub in range(NSUB):
                    nc.tensor.matmul(
                        ptr[:, sub * 128:(sub + 1) * 128],
                        phik[g][:, sub * 128:(sub + 1) * 128], idb_sb[:],
                        start=(sub == 0), stop=(sub == NSUB - 1))
                t = tm_p.tile([128, 512], bf16, tag=f"tm{g}")
                nc.scalar.copy(t[:], ptr[:])
                tmk.append(t)

            if STAGE == "tmk":
                ot = out_p.tile([128, C], f32, tag="out")
                nc.vector.tensor_copy(ot[:, :512], tmk[0][:])
                nc.sync.dma_start(out=outd[n0:n0 + 128, :], in_=ot[:])
                continue

            # ---- prefetch next chunk's x^T while the scan runs ----
            if cb + 1 < NBIG:
                xt_tiles = []
                for ct in range(6):
                    t = xt_p.tile([128, NCH], bf16, tag=f"xt{ct}")
                    nc.gpsimd.dma_start(
                        out=t[:],
                        in_=xT[ct * 128:(ct + 1) * 128,
                               (cb + 1) * NCH:(cb + 2) * NCH])
                    xt_tiles.append(t)

            # ---- causal scan over 128-token chunks ----
            # AT = phi_k @ phi_q^T.  Heads grouped by partition parity so
            # each PSUM bank group has ONE tile_position; one masked DVE
            # multiply per bank.  Emitted one sub-chunk AHEAD so the intra
            # matmuls never wait on the DVE multiply.
            def at_banks(s):
                sl = slice(s * 128, (s + 1) * 128)
                slot = {}
                for heads, e in (((0, 2, 4, 6), 0), ((8, 10), 0),
                                 ((1, 3, 5, 7), 1), ((9, 11), 1)):
                    nh_ = len(heads)
                    esl = slice(e * 64, (e + 1) * 64)
                    pa = pa_ps.tile([128, 512], f32, tag="pa")
                    for si, h in enumerate(heads):
                        g = h // 2
                        nc.tensor.matmul(
                            pa[:, si * 128:(si + 1) * 128],
                            phik[g][esl, sl], phiq[g][esl, sl],
                            start=(si == 0), stop=(si == nh_ - 1),
                            tile_position=(e * 64, 0))
                    atm = atm_p.tile([128, 512], bf16, tag="atm")
                    nc.vector.tensor_mul(
                        atm[:, :nh_ * 128].rearrange(
                            "p (h m) -> p h m", m=128),
                        pa[:, :nh_ * 128].rearrange("p (h m) -> p h m", m=128),
                        mk_sb[:].rearrange("p (o m) -> p o m", o=1)
                        .broadcast_to((128, nh_, 128)))
                    for si, h in enumerate(heads):
                        slot[h] = (atm, si)
                return slot

            atm_cur = at_banks(0)
            for sub in range(NSUB):
                ci = cb * NSUB + sub
                ssl = slice(sub * 128, (sub + 1) * 128)
                first = (ci == 0)
                last = (ci == NCHUNK - 1)
                nums = []
                for _grp in range(2):
                    pn = num_ps.tile([128, 512], f32, tag="num")
                    nums.append(pn)

                inter_on = STAGE == "full"
                state_on = STAGE in ("full", "scan_state")
                stbf_prev = stbf

                if STAGE == "scan_at":
                    ot = out_p.tile([128, C], f32, tag="out")
                    nc.vector.tensor_copy(ot[:, :512], atm_cur[0][0][:])
                    nc.sync.dma_start(
                        out=outd[n0 + sub * 128:n0 + (sub + 1) * 128, :],
                        in_=ot[:])
                    continue

                # state update: S += phi_k^T @ [V | 1] (PSUM-resident group
                # spanning all 32 chunks; per-chunk groups + SBUF fp32 chain
                # in the sim-safe variant)
                stc = st
                if SIM_SAFE:
                    stc = []
                    for b in range(2):
                        stc_t = st_ps.tile([128, 512], f32, tag=f"st{b}")
                        stc.append(stc_t)
                for g in range(G if state_on else 0):
                    bk, gg = g // 3, g % 3
                    nc.tensor.matmul(
                        stc[bk][:, gg * 130:gg * 130 + 130],
                        tmk[g][:, sub * 128:(sub + 1) * 128],
                        vsb[sub][:, 2 * g * 65:2 * g * 65 + 130],
                        start=(gg == 0 if SIM_SAFE else (first and gg == 0)),
                        stop=(gg == 2 if SIM_SAFE else (last and gg == 2)),
                        skip_group_check=True)
                if not last and state_on:
                    stbf_new = stbf_tiles[ci % 2]
                    if SIM_SAFE:
                        ssrc = []
                        for b in range(2):
                            stf_new = stf_p.tile([128, 390], f32,
                                                 tag=f"stf{b}")
                            if first:
                                nc.vector.tensor_copy(stf_new[:],
                                                      stc[b][:, :390])
                            else:
                                nc.vector.tensor_add(stf_new[:],
                                                     stc[b][:, :390],
                                                     stf[b][:])
                            stf[b] = stf_new
                            ssrc.append(stf_new)
                    else:
                        ssrc = st
                    for b in range(2):
                        for e in range(2):
                            esl = slice(e * 64, (e + 1) * 64)
                            csl = slice(e * 65, e * 65 + 65)
                            dst = stbf_new[esl, b * 390:(b + 1) * 390]                                .rearrange("p (g c) -> p g c", c=130)[:, :, csl]
                            nc.vector.tensor_copy(
                                dst, ssrc[b][esl, :390].rearrange(
                                    "p (g c) -> p g c", c=130)[:, :, csl])
                    stbf = stbf_new

                atm_slot = atm_cur
                if sub + 1 < NSUB:
                    atm_cur = at_banks(sub + 1)

                # inter-chunk: phi_q @ [S | z], one matmul per head pair
                # (zero-padded shadow keeps heads separate at K=128, and both
                # heads' 65-col blocks are adjacent in the num bank)
                if not first and inter_on:
                    for g in range(G):
                        nc.tensor.matmul(
                            nums[g // 3][:, (2 * g % 6) * 65:
                                         (2 * g % 6) * 65 + 130],
                            phiq[g][:, ssl],
                            stbf_prev[:, g * 130:g * 130 + 130],
                            start=(g % 3 == 0), stop=False)

                # intra-chunk: ATm^T @ [V | 1]  (closes each num bank)
                for h in range(H):
                    hh = h % 6
                    atm_t, si = atm_slot[h]
                    nc.tensor.matmul(
                        nums[h // 6][:, hh * 65:hh * 65 + 65],
                        atm_t[:, si * 128:(si + 1) * 128],
                        vsb[sub][:, h * 65:h * 65 + 65],
                        start=((first or not inter_on) and hh == 0),
                        stop=(hh == 5))

                if STAGE == "scan_intra":
                    ot = out_p.tile([128, C], f32, tag="out")
                    nc.scalar.copy(ot[:, :384], nums[0][:, :384])
                    nc.scalar.copy(ot[:, 384:768], nums[1][:, :384])
                    nc.sync.dma_start(
                        out=outd[n0 + sub * 128:n0 + (sub + 1) * 128, :],
                        in_=ot[:])
                    continue

                # ---- normalize and store ----
                den = den_p.tile([128, H], f32, tag="den")
                for grp in range(2):
                    src = nums[grp][:, :390].rearrange(
                        "p (h d) -> p h d", d=65)[:, :, 64:65]
                    dst = den[:, grp * 6:(grp + 1) * 6].rearrange(
                        "p (h o) -> p h o", o=1)
                    nc.vector.tensor_scalar_add(dst, src, 1e-16)
                nc.vector.reciprocal(den[:], den[:])
                ot = out_p.tile([128, C], f32, tag="out")
                for grp in range(2):
                    src = nums[grp][:, :390].rearrange(
                        "p (h d) -> p h d", d=65)[:, :, 0:64]
                    rec = den[:, grp * 6:(grp + 1) * 6].rearrange(
                        "p (h o) -> p h o", o=1).broadcast_to((128, 6, 64))
                    dst = ot[:, grp * 384:(grp + 1) * 384].rearrange(
                        "p (h d) -> p h d", d=64)
                    nc.vector.tensor_mul(dst, src, rec)
                nc.sync.dma_start(
                    out=outd[n0 + sub * 128:n0 + (sub + 1) * 128, :],
                    in_=ot[:])

    if not nc.is_finalized():
        nc.finalize()
    return nc


def _host_inputs(x, W_qkv, rfs):
    import ml_dtypes

    bf16 = ml_dtypes.bfloat16
    x = np.asarray(x, dtype=np.float32)
    W = np.asarray(W_qkv, dtype=np.float64)
    rfs = np.asarray(rfs, dtype=np.float64)

    # Fold scale*rfs into the q/k projection weights: P = Wphi @ x^T is then
    # the random-feature projection directly (rfs orthogonality gives
    # |z|^2 = |P|^2/64, recovered on-chip).
    Wq = W[:C].reshape(H, HD, C)
    Wk = W[C:2 * C].reshape(H, HD, C)
    Wphi_q = np.einsum('hlm,hlc->hmc', rfs * SCALE, Wq).reshape(C, C)
    Wphi_k = np.einsum('hlm,hlc->hmc', rfs * SCALE, Wk).reshape(C, C)
    Wall = np.concatenate([Wphi_q, Wphi_k, W[2 * C:]], axis=0)  # [3C, C]
    WT = np.ascontiguousarray(Wall.T).astype(bf16)              # [C, 3C]

    nhalf = np.zeros((128, 128), np.float32)
    nhalf[:64, :64] = -1.0 / (2 * HD)
    nhalf[64:, 64:] = -1.0 / (2 * HD)
    maskT = np.triu(np.ones((128, 128), np.float32))    # keep j <= i
    ident = np.eye(128, dtype=np.float32)

    shared = {"WT": WT, "nhalf": nhalf.astype(bf16),
              "maskT": maskT.astype(bf16), "ident": ident.astype(bf16)}
    in_maps = []
    for b in range(B):
        m = {"xT": np.ascontiguousarray(x[b].T).astype(bf16)}
        m.update(shared)
        in_maps.append(m)
    return in_maps


def kernel(x, W_qkv, rfs):
    from concourse.bass_utils import run_bass_kernel_spmd

    if "nc" not in _CACHE:
        _CACHE["nc"] = _build_bass()
    nc = _CACHE["nc"]
    in_maps = _host_inputs(x, W_qkv, rfs)
    res = run_bass_kernel_spmd(nc, in_maps, list(range(B)))
    return np.stack([res.results[b]["out"] for b in range(B)], axis=0)


# revision 29
# speedup vs baseline: 1.0347x; 1.0008x over previous
"""FAVOR+ (Performer) linear attention on 8 Trainium2 NeuronCores.

Math (per batch b, head h, with m = hd = 64, scale = hd**-0.25):
  qkv = x @ W_qkv.T ; q,k,v : [N, H, hd]
  phi(z) = exp(scale*z @ rfs[h] - 0.5*|scale*z|^2)          (z = q or k)
  causal scan:  S_t = S_{t-1} + phi_k[t] (x) v[t] ; z_t = z_{t-1} + phi_k[t]
                out[t] = (phi_q[t] @ S_t) / (phi_q[t] . z_t + 1e-16)

Sharding: data-parallel over batch B=8, one batch per core.

Key tricks vs the naive formulation:
  * rfs is orthogonal*sqrt(hd), so |z|^2 = |z @ rfs|^2 / hd.  This lets the
    host fold scale*rfs into W_q/W_k (P = Wphi @ x^T directly gives the
    random-feature projection) and the kernel recovers the -0.5|z|^2 term
    from P itself: exponent = P - |P|^2/(2*hd), via a block-diagonal
    (-1/128) matmul on P^2.
  * All matmul operands are bf16 (host-rounded; PSUM accumulation stays
    fp32), running the PE at full rate — fp32 operands cost 4 cycles/row.
  * v is stored interleaved with a ones column per head ([V|1], stride 65)
    so the intra-chunk numerator+denominator and the state update are one
    matmul each per head.
  * The scan state [S|z] accumulates in a single PSUM bank across all 32
    chunks (exact fp32); a bf16 SBUF shadow feeds the inter-chunk matmul.

Per-core chunked formulation (chunk L=128 tokens):
  AT   = phi_k_chunk @ phi_q_chunk^T          [j, i]  (PE, feature-major)
  ATm  = AT * triu_mask (keep j <= i)                 (DVE multiply, 4 heads/op)
  num' = phi_q @ [S | z]  +  ATm^T @ [V | 1]  [i, 65] (PE, PSUM-accumulated)
  S   += phi_k^T @ [V | 1]                            (PE, PSUM-resident)
  out  = num'[:, :64] / (num'[:, 64] + 1e-16)         (DVE recip + bcast mul)
"""

import os
import numpy as np

# sim-safe variant closes every PSUM group before reads (CoreSim rejects
# mid-accumulation-group PSUM reads; hardware does not care).  Slightly more
# DVE work.  Used to validate logic in CoreSim.
SIM_SAFE = bool(int(os.environ.get("KERNEL_SIM_SAFE", "0")))

# debug bisect stage: proj | vproj | tmk | scan_nostate | scan_state | full
STAGE = os.environ.get("KERNEL_STAGE", "full")

B, N, C, H = 8, 4096, 768, 12
HD = 64
G = H // 2            # head pairs stacked on 128 partitions
NCH = 512             # tokens per outer chunk
NSUB = NCH // 128     # 128-token scan chunks per outer chunk
NBIG = N // NCH
NCHUNK = N // 128     # 32 scan chunks
SCALE = HD ** -0.25

_CACHE = {}


def _build_bass():
    import concourse.bass as bass
    import concourse.mybir as mybir
    import concourse.tile as tile
    from concourse import bacc
    from contextlib import ExitStack

    f32 = mybir.dt.float32
    bf16 = mybir.dt.bfloat16
    AF = mybir.ActivationFunctionType

    nc = bacc.Bacc("TRN2", target_bir_lowering=False)
    xT = nc.declare_dram_parameter("xT", [C, N], bf16, isOutput=False)
    WT = nc.declare_dram_parameter("WT", [C, 3 * C], bf16, isOutput=False)
    nhalf = nc.declare_dram_parameter("nhalf", [128, 128], bf16, isOutput=False)
    maskT = nc.declare_dram_parameter("maskT", [128, 128], bf16, isOutput=False)
    ident = nc.declare_dram_parameter("ident", [128, 128], bf16, isOutput=False)
    outd = nc.declare_dram_parameter("out", [N, C], f32, isOutput=True)

    with tile.TileContext(nc) as tc, ExitStack() as ctx:
        consts = ctx.enter_context(tc.tile_pool(name="consts", bufs=1))
        xt_p = ctx.enter_context(tc.tile_pool(name="xt", bufs=2))
        sq_p = ctx.enter_context(tc.tile_pool(name="sq", bufs=2))
        phi_p = ctx.enter_context(tc.tile_pool(name="phi", bufs=2))
        tm_p = ctx.enter_context(tc.tile_pool(name="tm", bufs=2))
        v_p = ctx.enter_context(tc.tile_pool(name="v", bufs=2))
        atm_p = ctx.enter_context(tc.tile_pool(name="atm", bufs=8))
        stb_p = ctx.enter_context(tc.tile_pool(name="stb", bufs=1))
        stf_p = ctx.enter_context(tc.tile_pool(name="stf", bufs=2))
        den_p = ctx.enter_context(tc.tile_pool(name="den", bufs=4))
        out_p = ctx.enter_context(tc.tile_pool(name="outp", bufs=2))

        pp_ps = ctx.enter_context(tc.tile_pool(name="pp", bufs=2, space="PSUM"))
        num_ps = ctx.enter_context(tc.tile_pool(name="nm", bufs=2, space="PSUM"))
        pa_ps = ctx.enter_context(tc.tile_pool(name="pa", bufs=2, space="PSUM"))
        st_ps = ctx.enter_context(tc.tile_pool(name="stp", bufs=1, space="PSUM"))

        # ---- constants ----
        wt = []
        for ct in range(6):
            t = consts.tile([128, 3 * C], bf16, tag=f"wt{ct}")
            nc.sync.dma_start(out=t[:], in_=WT[ct * 128:(ct + 1) * 128, :])
            wt.append(t)
        nh_sb = consts.tile([128, 128], bf16, tag="nh")
        nc.sync.dma_start(out=nh_sb[:], in_=nhalf[:])
        mk_sb = consts.tile([128, 128], bf16, tag="mk")
        nc.sync.dma_start(out=mk_sb[:], in_=maskT[:])
        idb_sb = consts.tile([128, 128], bf16, tag="idb")
        nc.sync.dma_start(out=idb_sb[:], in_=ident[:])
        zb = consts.tile([128, 1], f32, tag="zb")
        nc.vector.memset(zb[:], 0.0)

        # PE warmup: dummy matmuls that run while the W/x DMAs stream in, so
        # the PE clock is fully ramped when the first projection starts.
        wrm = consts.tile([128, 64], bf16, tag="wrm")
        nc.vector.memset(wrm[:], 0.0)
        wps = pa_ps.tile([128, 512], f32, tag="pa")
        for _ in range(30):
            nc.tensor.matmul(wps[0:64, 0:64], wrm[:], wrm[:],
                             start=True, stop=True)

        # state [S|z] per head: pair g -> cols g*65..g*65+65, head parity e ->
        # partitions e*64..e*64+64.  One PSUM bank, accumulated over all 32
        # chunks; stbf is the bf16 SBUF shadow used by the inter-chunk matmul.
        st = None
        if not SIM_SAFE:
            st = []
            for b in range(2):
                stb_t = st_ps.tile([128, 512], f32, tag=f"st{b}")
                st.append(stb_t)
        stf = [None, None]  # fp32 SBUF state chain (sim-safe variant only)
        # state shadow, K=128 zero-padded: pair g cols g*130..g*130+130, head
        # parity e owns cols +e*65..+e*65+65 on partitions e*64..e*64+64, the
        # other parity's partitions stay zero so inter-chunk matmuls can run
        # with K=128 at tile_position (0,0) (uniform with the intra matmuls;
        # mixing row tile_positions inside one PSUM group faults the device).
        stbf_tiles = []
        for i in range(2):
            t = stb_p.tile([128, G * 130], bf16, tag=f"stb{i}")
            nc.vector.memset(t[:], 0.0)
            stbf_tiles.append(t)
        stbf = None

        # xt prefetch for chunk 0
        xt_tiles = []
        for ct in range(6):
            t = xt_p.tile([128, NCH], bf16, tag=f"xt{ct}")
            nc.gpsimd.dma_start(out=t[:], in_=xT[ct * 128:(ct + 1) * 128, 0:NCH])
            xt_tiles.append(t)

        for cb in range(NBIG):
            n0 = cb * NCH
            xt = xt_tiles

            # ---- P projection (feature-major random features for q,k) ----
            # P^T[f, n] accumulates in pf; then pf += nhalf @ (P*P) adds the
            # -|P|^2/128 exponent term; exp() gives phi^T in bf16.
            # Software-pipelined so the nhalf matmul (which waits on the Act
            # square) sits behind the next tile's projection matmuls.
            phi = [None] * 12
            pend = None  # (pf, sq, ft) awaiting nhalf+exp
            for ft in range(13):
                if ft < 12:
                    pf = pp_ps.tile([128, NCH], f32, tag="pp")
                    for ct in range(6):
                        nc.tensor.matmul(
                            pf[:], wt[ct][:, ft * 128:(ft + 1) * 128],
                            xt[ct][:],
                            start=(ct == 0), stop=(SIM_SAFE and ct == 5))
                    sq = sq_p.tile([128, NCH], bf16, tag="sqr")
                    nc.scalar.square(sq[:], pf[:])
                else:
                    pf = sq = None
                if pend is not None:
                    ppf, psq, pft = pend
                    if SIM_SAFE:
                        pn2 = pa_ps.tile([128, NCH], f32, tag="pa")
                        nc.tensor.matmul(pn2[:], nh_sb[:], psq[:],
                                         start=True, stop=True)
                        esum = stf_p.tile([128, NCH], f32, tag="esum")
                        nc.scalar.copy(esum[:], ppf[:])
                        nc.vector.tensor_add(esum[:], esum[:], pn2[:])
                        esrc = esum
                    else:
                        nc.tensor.matmul(ppf[:], nh_sb[:], psq[:],
                                         start=False, stop=True)
                        esrc = ppf
                    t = phi_p.tile([128, NCH], bf16, tag=f"ph{pft}")
                    nc.scalar.activation(t[:], esrc[:], AF.Exp, bias=zb[:])
                    phi[pft] = t
                pend = (pf, sq, ft) if ft < 12 else None
            phiq, phik = phi[:6], phi[6:]

            if STAGE == "proj":
                ot = out_p.tile([128, C], f32, tag="out")
                nc.vector.tensor_copy(ot[:, :512], phi[0][:])
                nc.vector.tensor_copy(ot[:, :512], phi[6][:])
                nc.sync.dma_start(out=outd[n0:n0 + 128, :], in_=ot[:])
                continue

            # ---- v projection (token-major), interleaved [V|1] stride 65 ----
            vsb = []
            for nt in range(NSUB):
                t = v_p.tile([128, H * 65], bf16, tag=f"v{nt}")
                ones_v = t[:].rearrange("p (h d) -> p h d", d=65)[:, :, 64:65]
                nc.vector.memset(ones_v, 1.0)
                for half in range(2):
                    pv = pp_ps.tile([128, NCH], f32, tag="pp")
                    fsl = slice(2 * C + half * 384, 2 * C + (half + 1) * 384)
                    for ct in range(6):
                        nc.tensor.matmul(
                            pv[:, :384], xt[ct][:, nt * 128:(nt + 1) * 128],
                            wt[ct][:, fsl], start=(ct == 0), stop=(ct == 5))
                    dst = t[:, half * 390:(half + 1) * 390].rearrange(
                        "p (h d) -> p h d", d=65)[:, :, 0:64]
                    src = pv[:, :384].rearrange("p (h d) -> p h d", d=64)
                    nc.scalar.copy(dst, src)
                vsb.append(t)

            if STAGE == "vproj":
                ot = out_p.tile([128, C], f32, tag="out")
                nc.vector.tensor_copy(ot[:], vsb[0][:, :C])
                nc.sync.dma_start(out=outd[n0:n0 + 128, :], in_=ot[:])
                continue

            # ---- phi_k token-major: transpose via regular matmul against the
            # identity (bf16 PE-transpose into PSUM is broken on HW; a plain
            # matmul phi_k^T @ I costs the same and lands in fp32 PSUM).
            # 4 sub-chunk transposes share one bank, one Act copy per pair.
            tmk = []
            for g in range(G):
                ptr = pp_ps.tile([128, 512], f32, tag="pp")
                for sub in range(NSUB):
                    nc.tensor.matmul(
                        ptr[:, sub * 128:(sub + 1) * 128],
                        phik[g][:, sub * 128:(sub + 1) * 128], idb_sb[:],
                        start=(sub == 0), stop=(sub == NSUB - 1))
                t = tm_p.tile([128, 512], bf16, tag=f"tm{g}")
                nc.scalar.copy(t[:], ptr[:])
                tmk.append(t)

            if STAGE == "tmk":
                ot = out_p.tile([128, C], f32, tag="out")
                nc.vector.tensor_copy(ot[:, :512], tmk[0][:])
                nc.sync.dma_start(out=outd[n0:n0 + 128, :], in_=ot[:])
                continue

            # ---- prefetch next chunk's x^T while the scan runs ----
            if cb + 1 < NBIG:
                xt_tiles = []
                for ct in range(6):
                    t = xt_p.tile([128, NCH], bf16, tag=f"xt{ct}")
                    nc.gpsimd.dma_start(
                        out=t[:],
                        in_=xT[ct * 128:(ct + 1) * 128,
                               (cb + 1) * NCH:(cb + 2) * NCH])
                    xt_tiles.append(t)

            # ---- causal scan over 128-token chunks ----
            # AT = phi_k @ phi_q^T.  Heads grouped by partition parity so
            # each PSUM bank group has ONE tile_position; one masked DVE
            # multiply per bank.  Emitted one sub-chunk AHEAD so the intra
            # matmuls never wait on the DVE multiply.
            def at_banks(s):
                sl = slice(s * 128, (s + 1) * 128)
                slot = {}
                for heads, e in (((0, 2, 4, 6), 0), ((8, 10), 0),
                                 ((1, 3, 5, 7), 1), ((9, 11), 1)):
                    nh_ = len(heads)
                    esl = slice(e * 64, (e + 1) * 64)
                    pa = pa_ps.tile([128, 512], f32, tag="pa")
                    for si, h in enumerate(heads):
                        g = h // 2
                        nc.tensor.matmul(
                            pa[:, si * 128:(si + 1) * 128],
                            phik[g][esl, sl], phiq[g][esl, sl],
                            start=(si == 0), stop=(si == nh_ - 1),
                            tile_position=(e * 64, 0))
                    atm = atm_p.tile([128, 512], bf16, tag="atm")
                    nc.vector.tensor_mul(
                        atm[:, :nh_ * 128].rearrange(
                            "p (h m) -> p h m", m=128),
                        pa[:, :nh_ * 128].rearrange("p (h m) -> p h m", m=128),
                        mk_sb[:].rearrange("p (o m) -> p o m", o=1)
                        .broadcast_to((128, nh_, 128)))
                    for si, h in enumerate(heads):
                        slot[h] = (atm, si)
                return slot

            atm_cur = at_banks(0)
            for sub in range(NSUB):
                ci = cb * NSUB + sub
                ssl = slice(sub * 128, (sub + 1) * 128)
                first = (ci == 0)
                last = (ci == NCHUNK - 1)
                nums = []
                for _grp in range(2):
                    pn = num_ps.tile([128, 512], f32, tag="num")
                    nums.append(pn)

                inter_on = STAGE == "full"
                state_on = STAGE in ("full", "scan_state")
                stbf_prev = stbf

                if STAGE == "scan_at":
                    ot = out_p.tile([128, C], f32, tag="out")
                    nc.vector.tensor_copy(ot[:, :512], atm_cur[0][0][:])
                    nc.sync.dma_start(
                        out=outd[n0 + sub * 128:n0 + (sub + 1) * 128, :],
                        in_=ot[:])
                    continue

                # state update: S += phi_k^T @ [V | 1] (PSUM-resident group
                # spanning all 32 chunks; per-chunk groups + SBUF fp32 chain
                # in the sim-safe variant)
                stc = st
                if SIM_SAFE:
                    stc = []
                    for b in range(2):
                        stc_t = st_ps.tile([128, 512], f32, tag=f"st{b}")
                        stc.append(stc_t)
                for g in range(G if state_on else 0):
                    bk, gg = g // 3, g % 3
                    nc.tensor.matmul(
                        stc[bk][:, gg * 130:gg * 130 + 130],
                        tmk[g][:, sub * 128:(sub + 1) * 128],
                        vsb[sub][:, 2 * g * 65:2 * g * 65 + 130],
                        start=(gg == 0 if SIM_SAFE else (first and gg == 0)),
                        stop=(gg == 2 if SIM_SAFE else (last and gg == 2)),
                        skip_group_check=True)
                if not last and state_on:
                    stbf_new = stbf_tiles[ci % 2]
                    if SIM_SAFE:
                        ssrc = []
                        for b in range(2):
                            stf_new = stf_p.tile([128, 390], f32,
                                                 tag=f"stf{b}")
                            if first:
                                nc.vector.tensor_copy(stf_new[:],
                                                      stc[b][:, :390])
                            else:
                                nc.vector.tensor_add(stf_new[:],
                                                     stc[b][:, :390],
                                                     stf[b][:])
                            stf[b] = stf_new
                            ssrc.append(stf_new)
                    else:
                        ssrc = st
                    for b in range(2):
                        for e in range(2):
                            esl = slice(e * 64, (e + 1) * 64)
                            csl = slice(e * 65, e * 65 + 65)
                            dst = stbf_new[esl, b * 390:(b + 1) * 390]                                .rearrange("p (g c) -> p g c", c=130)[:, :, csl]
                            nc.vector.tensor_copy(
                                dst, ssrc[b][esl, :390].rearrange(
                                    "p (g c) -> p g c", c=130)[:, :, csl])
                    stbf = stbf_new

                atm_slot = atm_cur
                if sub + 1 < NSUB:
                    atm_cur = at_banks(sub + 1)

                # inter-chunk: phi_q @ [S | z], one matmul per head pair
                # (zero-padded shadow keeps heads separate at K=128, and both
                # heads' 65-col blocks are adjacent in the num bank)
                if not first and inter_on:
                    for g in range(G):
                        nc.tensor.matmul(
                            nums[g // 3][:, (2 * g % 6) * 65:
                                         (2 * g % 6) * 65 + 130],
                            phiq[g][:, ssl],
                            stbf_prev[:, g * 130:g * 130 + 130],
                            start=(g % 3 == 0), stop=False)

                # intra-chunk: ATm^T @ [V | 1]  (closes each num bank)
                for h in range(H):
                    hh = h % 6
                    atm_t, si = atm_slot[h]
                    nc.tensor.matmul(
                        nums[h // 6][:, hh * 65:hh * 65 + 65],
                        atm_t[:, si * 128:(si + 1) * 128],
                        vsb[sub][:, h * 65:h * 65 + 65],
                        start=((first or not inter_on) and hh == 0),
                        stop=(hh == 5))

                if STAGE == "scan_intra":
                    ot = out_p.tile([128, C], f32, tag="out")
                    nc.scalar.copy(ot[:, :384], nums[0][:, :384])
                    nc.scalar.copy(ot[:, 384:768], nums[1][:, :384])
                    nc.sync.dma_start(
                        out=outd[n0 + sub * 128:n0 + (sub + 1) * 128, :],
                        in_=ot[:])
                    continue

                # ---- normalize and store ----
                den = den_p.tile([128, H], f32, tag="den")
                for grp in range(2):
                    src = nums[grp][:, :390].rearrange(
                        "p (h d) -> p h d", d=65)[:, :, 64:65]
                    dst = den[:, grp * 6:(grp + 1) * 6].rearrange(
                        "p (h o) -> p h o", o=1)
                    nc.vector.tensor_scalar_add(dst, src, 1e-16)
                nc.vector.reciprocal(den[:], den[:])
                ot = out_p.tile([128, C], f32, tag="out")
                for grp in range(2):
                    src = nums[grp][:, :390].rearrange(
                        "p (h d) -> p h d", d=65)[:, :, 0:64]
                    rec = den[:, grp * 6:(grp + 1) * 6].rearrange(
                        "p (h o) -> p h o", o=1).broadcast_to((128, 6, 64))
                    dst = ot[:, grp * 384:(grp + 1) * 384].rearrange(
                        "p (h d) -> p h d", d=64)
                    nc.vector.tensor_mul(dst, src, rec)
                nc.sync.dma_start(
                    out=outd[n0 + sub * 128:n0 + (sub + 1) * 128, :],
                    in_=ot[:])

    if not nc.is_finalized():
        nc.finalize()
    return nc


def _host_inputs(x, W_qkv, rfs):
    import ml_dtypes

    bf16 = ml_dtypes.bfloat16
    x = np.asarray(x, dtype=np.float32)
    W = np.asarray(W_qkv, dtype=np.float64)
    rfs = np.asarray(rfs, dtype=np.float64)

    # Fold scale*rfs into the q/k projection weights: P = Wphi @ x^T is then
    # the random-feature projection directly (rfs orthogonality gives
    # |z|^2 = |P|^2/64, recovered on-chip).
    Wq = W[:C].reshape(H, HD, C)
    Wk = W[C:2 * C].reshape(H, HD, C)
    Wphi_q = np.einsum('hlm,hlc->hmc', rfs * SCALE, Wq).reshape(C, C)
    Wphi_k = np.einsum('hlm,hlc->hmc', rfs * SCALE, Wk).reshape(C, C)
    Wall = np.concatenate([Wphi_q, Wphi_k, W[2 * C:]], axis=0)  # [3C, C]
    WT = np.ascontiguousarray(Wall.T).astype(bf16)              # [C, 3C]

    nhalf = np.zeros((128, 128), np.float32)
    nhalf[:64, :64] = -1.0 / (2 * HD)
    nhalf[64:, 64:] = -1.0 / (2 * HD)
    maskT = np.triu(np.ones((128, 128), np.float32))    # keep j <= i
    ident = np.eye(128, dtype=np.float32)

    shared = {"WT": WT, "nhalf": nhalf.astype(bf16),
              "maskT": maskT.astype(bf16), "ident": ident.astype(bf16)}
    in_maps = []
    for b in range(B):
        m = {"xT": np.ascontiguousarray(x[b].T).astype(bf16)}
        m.update(shared)
        in_maps.append(m)
    return in_maps


def kernel(x, W_qkv, rfs):
    from concourse.bass_utils import run_bass_kernel_spmd

    if "nc" not in _CACHE:
        _CACHE["nc"] = _build_bass()
    nc = _CACHE["nc"]
    in_maps = _host_inputs(x, W_qkv, rfs)
    res = run_bass_kernel_spmd(nc, in_maps, list(range(B)))
    return np.stack([res.results[b]["out"] for b in range(B)], axis=0)


# revision 31
# speedup vs baseline: 1.0495x; 1.0143x over previous
"""FAVOR+ (Performer) linear attention on 8 Trainium2 NeuronCores.

Math (per batch b, head h, with m = hd = 64, scale = hd**-0.25):
  qkv = x @ W_qkv.T ; q,k,v : [N, H, hd]
  phi(z) = exp(scale*z @ rfs[h] - 0.5*|scale*z|^2)          (z = q or k)
  causal scan:  S_t = S_{t-1} + phi_k[t] (x) v[t] ; z_t = z_{t-1} + phi_k[t]
                out[t] = (phi_q[t] @ S_t) / (phi_q[t] . z_t + 1e-16)

Sharding: data-parallel over batch B=8, one batch per core.

Key tricks vs the naive formulation:
  * rfs is orthogonal*sqrt(hd), so |z|^2 = |z @ rfs|^2 / hd.  This lets the
    host fold scale*rfs into W_q/W_k (P = Wphi @ x^T directly gives the
    random-feature projection) and the kernel recovers the -0.5|z|^2 term
    from P itself: exponent = P - |P|^2/(2*hd), via a block-diagonal
    (-1/128) matmul on P^2.
  * All matmul operands are bf16 (host-rounded; PSUM accumulation stays
    fp32), running the PE at full rate — fp32 operands cost 4 cycles/row.
  * v is stored interleaved with a ones column per head ([V|1], stride 65)
    so the intra-chunk numerator+denominator and the state update are one
    matmul each per head.
  * The scan state [S|z] accumulates in a single PSUM bank across all 32
    chunks (exact fp32); a bf16 SBUF shadow feeds the inter-chunk matmul.

Per-core chunked formulation (chunk L=128 tokens):
  AT   = phi_k_chunk @ phi_q_chunk^T          [j, i]  (PE, feature-major)
  ATm  = AT * triu_mask (keep j <= i)                 (DVE multiply, 4 heads/op)
  num' = phi_q @ [S | z]  +  ATm^T @ [V | 1]  [i, 65] (PE, PSUM-accumulated)
  S   += phi_k^T @ [V | 1]                            (PE, PSUM-resident)
  out  = num'[:, :64] / (num'[:, 64] + 1e-16)         (DVE recip + bcast mul)
"""

import os
import numpy as np

# sim-safe variant closes every PSUM group before reads (CoreSim rejects
# mid-accumulation-group PSUM reads; hardware does not care).  Slightly more
# DVE work.  Used to validate logic in CoreSim.
SIM_SAFE = bool(int(os.environ.get("KERNEL_SIM_SAFE", "0")))

# debug bisect stage: proj | vproj | tmk | scan_nostate | scan_state | full
STAGE = os.environ.get("KERNEL_STAGE", "full")

B, N, C, H = 8, 4096, 768, 12
HD = 64
G = H // 2            # head pairs stacked on 128 partitions
NCH = 512             # tokens per outer chunk
NSUB = NCH // 128     # 128-token scan chunks per outer chunk
NBIG = N // NCH
NCHUNK = N // 128     # 32 scan chunks
SCALE = HD ** -0.25

_CACHE = {}


def _build_bass():
    import concourse.bass as bass
    import concourse.mybir as mybir
    import concourse.tile as tile
    from concourse import bacc
    from contextlib import ExitStack

    f32 = mybir.dt.float32
    bf16 = mybir.dt.bfloat16
    AF = mybir.ActivationFunctionType

    nc = bacc.Bacc("TRN2", target_bir_lowering=False)
    xT = nc.declare_dram_parameter("xT", [C, N], bf16, isOutput=False)
    WT = nc.declare_dram_parameter("WT", [C, 3 * C], bf16, isOutput=False)
    nhalf = nc.declare_dram_parameter("nhalf", [128, 128], bf16, isOutput=False)
    maskT = nc.declare_dram_parameter("maskT", [128, 128], bf16, isOutput=False)
    ident = nc.declare_dram_parameter("ident", [128, 128], bf16, isOutput=False)
    outd = nc.declare_dram_parameter("out", [N, C], f32, isOutput=True)

    with tile.TileContext(nc) as tc, ExitStack() as ctx:
        consts = ctx.enter_context(tc.tile_pool(name="consts", bufs=1))
        xt_p = ctx.enter_context(tc.tile_pool(name="xt", bufs=2))
        sq_p = ctx.enter_context(tc.tile_pool(name="sq", bufs=2))
        phi_p = ctx.enter_context(tc.tile_pool(name="phi", bufs=2))
        tm_p = ctx.enter_context(tc.tile_pool(name="tm", bufs=2))
        v_p = ctx.enter_context(tc.tile_pool(name="v", bufs=2))
        atm_p = ctx.enter_context(tc.tile_pool(name="atm", bufs=8))
        stb_p = ctx.enter_context(tc.tile_pool(name="stb", bufs=1))
        stf_p = ctx.enter_context(tc.tile_pool(name="stf", bufs=2))
        den_p = ctx.enter_context(tc.tile_pool(name="den", bufs=4))
        out_p = ctx.enter_context(tc.tile_pool(name="outp", bufs=2))

        pp_ps = ctx.enter_context(tc.tile_pool(name="pp", bufs=2, space="PSUM"))
        num_ps = ctx.enter_context(tc.tile_pool(name="nm", bufs=2, space="PSUM"))
        pa_ps = ctx.enter_context(tc.tile_pool(name="pa", bufs=2, space="PSUM"))
        st_ps = ctx.enter_context(tc.tile_pool(name="stp", bufs=1, space="PSUM"))

        # ---- constants ----
        # W tiles spread across three DMA queues so the projection is not
        # serialized behind a single ~10us W stream at startup; x goes first
        # on the gpsimd queue so the ct=0 matmul can start immediately.
        wt = []
        wt_q = [nc.sync, nc.scalar, nc.sync, nc.scalar, None, None]
        for ct in range(6):
            t = consts.tile([128, 3 * C], bf16, tag=f"wt{ct}")
            if wt_q[ct] is not None:
                wt_q[ct].dma_start(out=t[:], in_=WT[ct * 128:(ct + 1) * 128, :])
            wt.append(t)
        nh_sb = consts.tile([128, 128], bf16, tag="nh")
        nc.scalar.dma_start(out=nh_sb[:], in_=nhalf[:])
        mk_sb = consts.tile([128, 128], bf16, tag="mk")
        nc.scalar.dma_start(out=mk_sb[:], in_=maskT[:])
        idb_sb = consts.tile([128, 128], bf16, tag="idb")
        nc.scalar.dma_start(out=idb_sb[:], in_=ident[:])
        zb = consts.tile([128, 1], f32, tag="zb")
        nc.vector.memset(zb[:], 0.0)

        # PE warmup: dummy matmuls that run while the W/x DMAs stream in, so
        # the PE clock is fully ramped when the first projection starts.
        wrm = consts.tile([128, 64], bf16, tag="wrm")
        nc.vector.memset(wrm[:], 0.0)
        wps = pa_ps.tile([128, 512], f32, tag="pa")
        for _ in range(30):
            nc.tensor.matmul(wps[0:64, 0:64], wrm[:], wrm[:],
                             start=True, stop=True)

        # state [S|z] per head: pair g -> cols g*65..g*65+65, head parity e ->
        # partitions e*64..e*64+64.  One PSUM bank, accumulated over all 32
        # chunks; stbf is the bf16 SBUF shadow used by the inter-chunk matmul.
        st = None
        if not SIM_SAFE:
            st = []
            for b in range(2):
                stb_t = st_ps.tile([128, 512], f32, tag=f"st{b}")
                st.append(stb_t)
        stf = [None, None]  # fp32 SBUF state chain (sim-safe variant only)
        # state shadow, K=128 zero-padded: pair g cols g*130..g*130+130, head
        # parity e owns cols +e*65..+e*65+65 on partitions e*64..e*64+64, the
        # other parity's partitions stay zero so inter-chunk matmuls can run
        # with K=128 at tile_position (0,0) (uniform with the intra matmuls;
        # mixing row tile_positions inside one PSUM group faults the device).
        stbf_tiles = []
        for i in range(2):
            t = stb_p.tile([128, G * 130], bf16, tag=f"stb{i}")
            nc.vector.memset(t[:], 0.0)
            stbf_tiles.append(t)
        stbf = None

        # xt prefetch for chunk 0, then the remaining W tiles on this queue
        xt_tiles = []
        for ct in range(6):
            t = xt_p.tile([128, NCH], bf16, tag=f"xt{ct}")
            nc.gpsimd.dma_start(out=t[:], in_=xT[ct * 128:(ct + 1) * 128, 0:NCH])
            xt_tiles.append(t)
        for ct in (4, 5):
            nc.gpsimd.dma_start(out=wt[ct][:],
                                in_=WT[ct * 128:(ct + 1) * 128, :])

        for cb in range(NBIG):
            n0 = cb * NCH
            xt = xt_tiles

            # ---- P projection (feature-major random features for q,k) ----
            # P^T[f, n] accumulates in pf; then pf += nhalf @ (P*P) adds the
            # -|P|^2/128 exponent term; exp() gives phi^T in bf16.
            # Software-pipelined so the nhalf matmul (which waits on the Act
            # square) sits behind the next tile's projection matmuls.
            phi = [None] * 12
            pend = None  # (pf, sq, ft) awaiting nhalf+exp
            for ft in range(13):
                if ft < 12:
                    pf = pp_ps.tile([128, NCH], f32, tag="pp")
                    for ct in range(6):
                        nc.tensor.matmul(
                            pf[:], wt[ct][:, ft * 128:(ft + 1) * 128],
                            xt[ct][:],
                            start=(ct == 0), stop=(SIM_SAFE and ct == 5))
                    sq = sq_p.tile([128, NCH], bf16, tag="sqr")
                    nc.scalar.square(sq[:], pf[:])
                else:
                    pf = sq = None
                if pend is not None:
                    ppf, psq, pft = pend
                    if SIM_SAFE:
                        pn2 = pa_ps.tile([128, NCH], f32, tag="pa")
                        nc.tensor.matmul(pn2[:], nh_sb[:], psq[:],
                                         start=True, stop=True)
                        esum = stf_p.tile([128, NCH], f32, tag="esum")
                        nc.scalar.copy(esum[:], ppf[:])
                        nc.vector.tensor_add(esum[:], esum[:], pn2[:])
                        esrc = esum
                    else:
                        nc.tensor.matmul(ppf[:], nh_sb[:], psq[:],
                                         start=False, stop=True)
                        esrc = ppf
                    t = phi_p.tile([128, NCH], bf16, tag=f"ph{pft}")
                    nc.scalar.activation(t[:], esrc[:], AF.Exp, bias=zb[:])
                    phi[pft] = t
                pend = (pf, sq, ft) if ft < 12 else None
            phiq, phik = phi[:6], phi[6:]

            if STAGE == "proj":
                ot = out_p.tile([128, C], f32, tag="out")
                nc.vector.tensor_copy(ot[:, :512], phi[0][:])
                nc.vector.tensor_copy(ot[:, :512], phi[6][:])
                nc.sync.dma_start(out=outd[n0:n0 + 128, :], in_=ot[:])
                continue

            # ---- v projection (token-major), interleaved [V|1] stride 65 ----
            vsb = []
            for nt in range(NSUB):
                t = v_p.tile([128, H * 65], bf16, tag=f"v{nt}")
                ones_v = t[:].rearrange("p (h d) -> p h d", d=65)[:, :, 64:65]
                nc.vector.memset(ones_v, 1.0)
                for half in range(2):
                    pv = pp_ps.tile([128, NCH], f32, tag="pp")
                    fsl = slice(2 * C + half * 384, 2 * C + (half + 1) * 384)
                    for ct in range(6):
                        nc.tensor.matmul(
                            pv[:, :384], xt[ct][:, nt * 128:(nt + 1) * 128],
                            wt[ct][:, fsl], start=(ct == 0), stop=(ct == 5))
                    dst = t[:, half * 390:(half + 1) * 390].rearrange(
                        "p (h d) -> p h d", d=65)[:, :, 0:64]
                    src = pv[:, :384].rearrange("p (h d) -> p h d", d=64)
                    nc.scalar.copy(dst, src)
                vsb.append(t)

            if STAGE == "vproj":
                ot = out_p.tile([128, C], f32, tag="out")
                nc.vector.tensor_copy(ot[:], vsb[0][:, :C])
                nc.sync.dma_start(out=outd[n0:n0 + 128, :], in_=ot[:])
                continue

            # ---- phi_k token-major: transpose via regular matmul against the
            # identity (bf16 PE-transpose into PSUM is broken on HW; a plain
            # matmul phi_k^T @ I costs the same and lands in fp32 PSUM).
            # 4 sub-chunk transposes share one bank, one Act copy per pair.
            tmk = []
            for g in range(G):
                ptr = pp_ps.tile([128, 512], f32, tag="pp")
                for sub in range(NSUB):
                    nc.tensor.matmul(
                        ptr[:, sub * 128:(sub + 1) * 128],
                        phik[g][:, sub * 128:(sub + 1) * 128], idb_sb[:],
                        start=(sub == 0), stop=(sub == NSUB - 1))
                t = tm_p.tile([128, 512], bf16, tag=f"tm{g}")
                nc.scalar.copy(t[:], ptr[:])
                tmk.append(t)

            if STAGE == "tmk":
                ot = out_p.tile([128, C], f32, tag="out")
                nc.vector.tensor_copy(ot[:, :512], tmk[0][:])
                nc.sync.dma_start(out=outd[n0:n0 + 128, :], in_=ot[:])
                continue

            # ---- prefetch next chunk's x^T while the scan runs ----
            if cb + 1 < NBIG:
                xt_tiles = []
                for ct in range(6):
                    t = xt_p.tile([128, NCH], bf16, tag=f"xt{ct}")
                    nc.gpsimd.dma_start(
                        out=t[:],
                        in_=xT[ct * 128:(ct + 1) * 128,
                               (cb + 1) * NCH:(cb + 2) * NCH])
                    xt_tiles.append(t)

            # ---- causal scan over 128-token chunks ----
            # AT = phi_k @ phi_q^T.  Heads grouped by partition parity so
            # each PSUM bank group has ONE tile_position; one masked DVE
            # multiply per bank.  Emitted one sub-chunk AHEAD so the intra
            # matmuls never wait on the DVE multiply.
            def at_banks(s):
                sl = slice(s * 128, (s + 1) * 128)
                slot = {}
                for heads, e in (((0, 2, 4, 6), 0), ((8, 10), 0),
                                 ((1, 3, 5, 7), 1), ((9, 11), 1)):
                    nh_ = len(heads)
                    esl = slice(e * 64, (e + 1) * 64)
                    pa = pa_ps.tile([128, 512], f32, tag="pa")
                    for si, h in enumerate(heads):
                        g = h // 2
                        nc.tensor.matmul(
                            pa[:, si * 128:(si + 1) * 128],
                            phik[g][esl, sl], phiq[g][esl, sl],
                            start=(si == 0), stop=(si == nh_ - 1),
                            tile_position=(e * 64, 0))
                    atm = atm_p.tile([128, 512], bf16, tag="atm")
                    nc.vector.tensor_mul(
                        atm[:, :nh_ * 128].rearrange(
                            "p (h m) -> p h m", m=128),
                        pa[:, :nh_ * 128].rearrange("p (h m) -> p h m", m=128),
                        mk_sb[:].rearrange("p (o m) -> p o m", o=1)
                        .broadcast_to((128, nh_, 128)))
                    for si, h in enumerate(heads):
                        slot[h] = (atm, si)
                return slot

            atm_cur = at_banks(0)
            for sub in range(NSUB):
                ci = cb * NSUB + sub
                ssl = slice(sub * 128, (sub + 1) * 128)
                first = (ci == 0)
                last = (ci == NCHUNK - 1)
                nums = []
                for _grp in range(2):
                    pn = num_ps.tile([128, 512], f32, tag="num")
                    nums.append(pn)

                inter_on = STAGE == "full"
                state_on = STAGE in ("full", "scan_state")
                stbf_prev = stbf

                if STAGE == "scan_at":
                    ot = out_p.tile([128, C], f32, tag="out")
                    nc.vector.tensor_copy(ot[:, :512], atm_cur[0][0][:])
                    nc.sync.dma_start(
                        out=outd[n0 + sub * 128:n0 + (sub + 1) * 128, :],
                        in_=ot[:])
                    continue

                # state update: S += phi_k^T @ [V | 1] (PSUM-resident group
                # spanning all 32 chunks; per-chunk groups + SBUF fp32 chain
                # in the sim-safe variant)
                stc = st
                if SIM_SAFE:
                    stc = []
                    for b in range(2):
                        stc_t = st_ps.tile([128, 512], f32, tag=f"st{b}")
                        stc.append(stc_t)
                for g in range(G if state_on else 0):
                    bk, gg = g // 3, g % 3
                    nc.tensor.matmul(
                        stc[bk][:, gg * 130:gg * 130 + 130],
                        tmk[g][:, sub * 128:(sub + 1) * 128],
                        vsb[sub][:, 2 * g * 65:2 * g * 65 + 130],
                        start=(gg == 0 if SIM_SAFE else (first and gg == 0)),
                        stop=(gg == 2 if SIM_SAFE else (last and gg == 2)),
                        skip_group_check=True)
                if not last and state_on:
                    stbf_new = stbf_tiles[ci % 2]
                    if SIM_SAFE:
                        ssrc = []
                        for b in range(2):
                            stf_new = stf_p.tile([128, 390], f32,
                                                 tag=f"stf{b}")
                            if first:
                                nc.vector.tensor_copy(stf_new[:],
                                                      stc[b][:, :390])
                            else:
                                nc.vector.tensor_add(stf_new[:],
                                                     stc[b][:, :390],
                                                     stf[b][:])
                            stf[b] = stf_new
                            ssrc.append(stf_new)
                    else:
                        ssrc = st
                    for b in range(2):
                        for e in range(2):
                            esl = slice(e * 64, (e + 1) * 64)
                            csl = slice(e * 65, e * 65 + 65)
                            dst = stbf_new[esl, b * 390:(b + 1) * 390]                                .rearrange("p (g c) -> p g c", c=130)[:, :, csl]
                            nc.vector.tensor_copy(
                                dst, ssrc[b][esl, :390].rearrange(
                                    "p (g c) -> p g c", c=130)[:, :, csl])
                    stbf = stbf_new

                atm_slot = atm_cur
                if sub + 1 < NSUB:
                    atm_cur = at_banks(sub + 1)

                # inter-chunk: phi_q @ [S | z], one matmul per head pair
                # (zero-padded shadow keeps heads separate at K=128, and both
                # heads' 65-col blocks are adjacent in the num bank)
                if not first and inter_on:
                    for g in range(G):
                        nc.tensor.matmul(
                            nums[g // 3][:, (2 * g % 6) * 65:
                                         (2 * g % 6) * 65 + 130],
                            phiq[g][:, ssl],
                            stbf_prev[:, g * 130:g * 130 + 130],
                            start=(g % 3 == 0), stop=False)

                # intra-chunk: ATm^T @ [V | 1]  (closes each num bank)
                for h in range(H):
                    hh = h % 6
                    atm_t, si = atm_slot[h]
                    nc.tensor.matmul(
                        nums[h // 6][:, hh * 65:hh * 65 + 65],
                        atm_t[:, si * 128:(si + 1) * 128],
                        vsb[sub][:, h * 65:h * 65 + 65],
                        start=((first or not inter_on) and hh == 0),
                        stop=(hh == 5))

                if STAGE == "scan_intra":
                    ot = out_p.tile([128, C], f32, tag="out")
                    nc.scalar.copy(ot[:, :384], nums[0][:, :384])
                    nc.scalar.copy(ot[:, 384:768], nums[1][:, :384])
                    nc.sync.dma_start(
                        out=outd[n0 + sub * 128:n0 + (sub + 1) * 128, :],
                        in_=ot[:])
                    continue

                # ---- normalize and store ----
                den = den_p.tile([128, H], f32, tag="den")
                for grp in range(2):
                    src = nums[grp][:, :390].rearrange(
                        "p (h d) -> p h d", d=65)[:, :, 64:65]
                    dst = den[:, grp * 6:(grp + 1) * 6].rearrange(
                        "p (h o) -> p h o", o=1)
                    nc.vector.tensor_scalar_add(dst, src, 1e-16)
                nc.vector.reciprocal(den[:], den[:])
                ot = out_p.tile([128, C], f32, tag="out")
                for grp in range(2):
                    src = nums[grp][:, :390].rearrange(
                        "p (h d) -> p h d", d=65)[:, :, 0:64]
                    rec = den[:, grp * 6:(grp + 1) * 6].rearrange(
                        "p (h o) -> p h o", o=1).broadcast_to((128, 6, 64))
                    dst = ot[:, grp * 384:(grp + 1) * 384].rearrange(
                        "p (h d) -> p h d", d=64)
                    nc.vector.tensor_mul(dst, src, rec)
                nc.sync.dma_start(
                    out=outd[n0 + sub * 128:n0 + (sub + 1) * 128, :],
                    in_=ot[:])

    if not nc.is_finalized():
        nc.finalize()
    return nc


def _host_inputs(x, W_qkv, rfs):
    import ml_dtypes

    bf16 = ml_dtypes.bfloat16
    x = np.asarray(x, dtype=np.float32)
    W = np.asarray(W_qkv, dtype=np.float64)
    rfs = np.asarray(rfs, dtype=np.float64)

    # Fold scale*rfs into the q/k projection weights: P = Wphi @ x^T is then
    # the random-feature projection directly (rfs orthogonality gives
    # |z|^2 = |P|^2/64, recovered on-chip).
    Wq = W[:C].reshape(H, HD, C)
    Wk = W[C:2 * C].reshape(H, HD, C)
    Wphi_q = np.einsum('hlm,hlc->hmc', rfs * SCALE, Wq).reshape(C, C)
    Wphi_k = np.einsum('hlm,hlc->hmc', rfs * SCALE, Wk).reshape(C, C)
    Wall = np.concatenate([Wphi_q, Wphi_k, W[2 * C:]], axis=0)  # [3C, C]
    WT = np.ascontiguousarray(Wall.T).astype(bf16)              # [C, 3C]

    nhalf = np.zeros((128, 128), np.float32)
    nhalf[:64, :64] = -1.0 / (2 * HD)
    nhalf[64:, 64:] = -1.0 / (2 * HD)
    maskT = np.triu(np.ones((128, 128), np.float32))    # keep j <= i
    ident = np.eye(128, dtype=np.float32)

    shared = {"WT": WT, "nhalf": nhalf.astype(bf16),
              "maskT": maskT.astype(bf16), "ident": ident.astype(bf16)}
    in_maps = []
    for b in range(B):
        m = {"xT": np.ascontiguousarray(x[b].T).astype(bf16)}
        m.update(shared)
        in_maps.append(m)
    return in_maps


def kernel(x, W_qkv, rfs):
    from concourse.bass_utils import run_bass_kernel_spmd

    if "nc" not in _CACHE:
        _CACHE["nc"] = _build_bass()
    nc = _CACHE["nc"]
    in_maps = _host_inputs(x, W_qkv, rfs)
    res = run_bass_kernel_spmd(nc, in_maps, list(range(B)))
    return np.stack([res.results[b]["out"] for b in range(B)], axis=0)
